# revision 21
# baseline (speedup 1.0000x reference)
"""GCN regressor (3x GCNConv + BatchNorm + ReLU) on 8 Trainium2 NeuronCores.

Sharding (graph/data parallel, per the hint):
  - Nodes are split into 8 contiguous blocks of 6250; a core owns the dsts in
    its block and all edges pointing at them. Within a core, dsts are laid out
    rank i -> (lane p=i%128, slot t=i//128) in a [128, 49*64] layout (padded
    to 6272 rows); the rank order is chosen on host to minimize gather padding
    (window-sort by per-half in-degree).
  - Per layer the dinv-prescaled feature table (all 50176 padded rows) is
    AllGathered into every core's DRAM (the "halo exchange").
  - The sparse aggregation uses the SWDGE dma_gather custom instruction
    (GPSIMD mlp library): each instruction gathers up to 64*128 rows by int16
    index, amortizing the ~1us per-instruction SWDGE overhead over thousands
    of descriptors (~30 instructions/layer instead of 1600). int16 indices cap
    the addressable table at 32768 rows, so the 50176-row table is split in
    two halves (cores 0-4 / 5-7) with per-half gather streams; each lane's
    edges are split by source half, exhausted lanes point at a zero row.
  - Features move as fp16: the halo AllGather exchanges compact fp16 rows
    (half the bytes), locally re-strided to 256B rows (the gather ISA's
    stride granularity) whose 128B pad half is never read or transferred.
  - Gathered rows are reduced per destination slot by identity-matmul
    accumulation in f32 PSUM on the otherwise-idle PE engine; the self-loop
    term is folded in as one extra matmul; the ACT engine drains PSUM with
    the dinv output scale. The per-slot transposes and weight matmuls are
    interleaved one slot behind the drain so the whole dense chain hides
    under the gather pipeline.
  - Dense math (x@W, BN, ReLU) runs on PE/DVE/ACT; BN statistics are
    AllReduced (biased variance, as the reference).
  - b1/b2 are absorbed by BatchNorm (BN(z+b)=BN(z)) and unused.

kernel(**inputs) takes FULL inputs, returns the FULL [50000] output (f32).
"""

import sys

sys.path.insert(0, '/opt/trn_rl_repo')

import numpy as np

import concourse.bass as bass
import concourse.bacc as bacc
import concourse.tile as tile
import concourse.mybir as mybir
from concourse import library_config
from concourse.masks import make_identity


def _patch_dma_gather():
    """Allow 128-byte gather payloads (elem_step stays 256B-granular, which
    the InstDMAGatherAnt stride_bytes_256 ISA field requires)."""
    import inspect, textwrap
    src = textwrap.dedent(inspect.getsource(bass.BassGpSimd.dma_gather))
    src = src.replace("""    assert (
        elem_size_bytes > 0 and elem_size_bytes % 256 == 0
    )  # transpose restriction""", "    assert elem_size_bytes > 0")
    g = dict(bass.BassGpSimd.dma_gather.__globals__)
    exec(src, g)
    bass.BassGpSimd.dma_gather = g["dma_gather"]


_patch_dma_gather()

F32 = mybir.dt.float32
F16 = mybir.dt.float16
I16 = mybir.dt.int16
AF = mybir.ActivationFunctionType

D = 64
NC = 8
EPS = 1e-5
H0_CORES = 5        # table halves: cores 0..4 / 5..7 (int16 idx < 32768)
WSORT = 768         # window size for the lane-order heuristic
MAXC = 64           # gather columns per dma_gather (single_packet=False
                    # lifts the 1024-idx single-packet ucode cap)
WA = 128            # A-half stage window (columns, multiple of MAXC)
WB = 64             # B-half stage window (columns)


class Cfg:
    def __init__(self, n_nodes, n_cores=NC):
        self.n = n_nodes
        self.nc = n_cores
        self.nloc = n_nodes // n_cores
        assert self.nloc * n_cores == n_nodes
        self.slots = self.nloc // 128 + 1          # ensures pad ranks exist
        self.npad = self.slots * 128
        assert self.nloc < self.npad
        self.ntab = self.npad * n_cores
        self.h0_rows = H0_CORES * self.npad
        self.h1_rows = self.ntab - self.h0_rows
        assert self.h0_rows < 32768 and self.h1_rows < 32768


class Sched:
    """Gather schedule: per-slot column counts and packed idx layout."""
    def __init__(self, ca, cb, chunks, icols):
        self.ca = ca            # [S] A-columns (H0) per slot
        self.cb = cb            # [S] B-columns (H1) per slot
        self.chunks = chunks    # list of (half, icol0, ncols) in issue order
        self.icols = icols      # total idx-tile int16 columns
        self.key = (tuple(ca), tuple(cb), tuple(chunks))


def host_prep(cfg, edge_index):
    """Degrees, per-core lane ordering, per-half gather idx streams."""
    n, nc_, nloc, npad, S = cfg.n, cfg.nc, cfg.nloc, cfg.npad, cfg.slots
    src = np.asarray(edge_index[0], dtype=np.int64)
    dst = np.asarray(edge_index[1], dtype=np.int64)
    deg = np.bincount(dst, minlength=n).astype(np.int64) + 1   # + self loop

    is_h1 = (src // nloc) >= H0_CORES
    e1 = np.bincount(dst[is_h1], minlength=n)
    e0 = np.bincount(dst[~is_h1], minlength=n)

    # lane order: sort by e0 desc in windows, e1 desc within each window
    perms = []
    rank = np.zeros(n, dtype=np.int64)
    for c in range(nc_):
        nodes = np.arange(c * nloc, (c + 1) * nloc)
        o = np.argsort(-e0[nodes], kind="stable")
        parts = []
        for s in range(0, nloc, WSORT):
            blk = o[s:s + WSORT]
            parts.append(blk[np.argsort(-e1[nodes[blk]], kind="stable")])
        p = nodes[np.concatenate(parts)]
        perms.append(p)
        rank[p] = np.arange(nloc)

    # table row of node m: core*npad + (rank%128)*S + rank//128
    g_of = (np.int64(npad) * (np.arange(n) // nloc)
            + (rank % 128) * S + rank // 128)
    zrel = 127 * S + (S - 1)   # pad row (rank npad-1) of a block, block-rel

    # global column counts (shared across cores so one program fits all)
    ca = np.zeros(S, dtype=np.int64)
    cb = np.zeros(S, dtype=np.int64)
    lane_lists = []   # per core: dict (t, p) -> (listA, listB) of table rows
    for c in range(nc_):
        m = (dst // nloc) == c
        s_c, d_c = src[m], dst[m]
        r_c = rank[d_c]
        h1_c = is_h1[m]
        order = np.lexsort((g_of[s_c], h1_c, r_c))  # rank, half, src row
        s_c, r_c, h1_c = s_c[order], r_c[order], h1_c[order]
        rows = g_of[s_c]
        # counts per rank and half
        cnt0 = np.bincount(r_c[~h1_c], minlength=npad)
        cnt1 = np.bincount(r_c[h1_c], minlength=npad)
        for t in range(S):
            ca[t] = max(ca[t], cnt0[t * 128:(t + 1) * 128].max())
            cb[t] = max(cb[t], cnt1[t * 128:(t + 1) * 128].max())
        lane_lists.append((rows[~h1_c], r_c[~h1_c], cnt0,
                           rows[h1_c] - cfg.h0_rows, r_c[h1_c], cnt1))

    # Per-half flat column streams (A then B), in slot order. Chunks are
    # fixed MAXC-column (128*MAXC-idx, multi-packet) pieces of each stream;
    # stage windows are WA/WB columns, multiples of MAXC for the A half and
    # exactly MAXC for the B half, so chunks never straddle windows.
    chunks = []            # (half, icol0, ncols, stream_col0)
    icol = 0
    for half, cc in ((0, ca), (1, cb)):
        ncols = int(cc.sum())
        done = 0
        while done < ncols:
            m = min(MAXC, ncols - done)
            chunks.append((half, icol, m, done))
            icol += m * 8          # 128 idx/col / 16 partitions
            done += m

    idxs = np.zeros((nc_, 128, max(icol, 1)), dtype=np.int16)
    for c in range(nc_):
        rowsA, rA, cnt0, rowsB, rB, cnt1 = lane_lists[c]
        startsA = np.concatenate([[0], np.cumsum(cnt0)])
        startsB = np.concatenate([[0], np.cumsum(cnt1)])
        flat_cols = []     # int16 [128] per column: A stream then B stream
        for cc, rows_, starts, cnt in ((ca, rowsA, startsA, cnt0),
                                       (cb, rowsB, startsB, cnt1)):
            for t in range(S):
                ranks = t * 128 + np.arange(128)
                for k in range(int(cc[t])):
                    col = np.full(128, zrel, dtype=np.int64)
                    have = cnt[ranks] > k
                    col[have] = rows_[starts[ranks[have]] + k]
                    flat_cols.append(col.astype(np.int16))
        if flat_cols:
            fc = np.stack(flat_cols)            # [ncols, 128]
            flat = fc.reshape(-1)               # position k = col*128 + p
            wrapped = np.zeros((16, icol), np.int16)
            kk = np.arange(len(flat))
            wrapped[kk % 16, kk // 16] = flat
            idxs[c] = np.tile(wrapped, (8, 1))
    sched = Sched([int(x) for x in ca], [int(x) for x in cb], chunks,
                  max(icol, 1))
    return deg, perms, sched, idxs


def build(cfg, sched):
    nc = bacc.Bacc("TRN2", target_bir_lowering=False, debug=False,
                   enable_asserts=False, num_devices=cfg.nc,
                   num_swdge_queues=4)
    S = cfg.slots
    NPF = S * 64
    NT = cfg.ntab
    NP = cfg.npad
    ca, cb = sched.ca, sched.cb

    xl = nc.dram_tensor("xl", [128, NPF], F32, kind="ExternalInput").ap()
    degt = nc.dram_tensor("degt", [128, S], F32, kind="ExternalInput").ap()
    idx = nc.dram_tensor("idx", [128, sched.icols], I16,
                         kind="ExternalInput").ap()
    w1 = nc.dram_tensor("w1", [D, D], F32, kind="ExternalInput").ap()
    w2 = nc.dram_tensor("w2", [D, D], F32, kind="ExternalInput").ap()
    w3 = nc.dram_tensor("w3", [D, 1], F32, kind="ExternalInput").ap()
    gb = nc.dram_tensor("gb", [4, D], F32, kind="ExternalInput").ap()
    b3 = nc.dram_tensor("b3", [1, 1], F32, kind="ExternalInput").ap()
    out = nc.dram_tensor("out", [1, NP], F32, kind="ExternalOutput").ap()

    inv_n = 1.0 / cfg.n
    chunks_ab = ([c for c in sched.chunks if c[0] == 0],
                 [c for c in sched.chunks if c[0] == 1])

    with tile.TileContext(nc) as tc:
        with tc.tile_pool(name="const", bufs=1) as cpool, \
             tc.tile_pool(name="stga", bufs=2) as spool_a, \
             tc.tile_pool(name="stgb", bufs=2) as spool_b, \
             tc.tile_pool(name="osml", bufs=2) as opool, \
             tc.tile_pool(name="ps", bufs=2, space="PSUM") as ppool, \
             tc.tile_pool(name="acm", bufs=2, space="PSUM") as apool, \
             tc.tile_pool(name="ps1", bufs=1, space="PSUM") as ppool1, \
             tc.tile_pool(name="dram", bufs=1, space="DRAM") as dpool:

            nc.gpsimd.load_library(library_config.mlp)

            ident = cpool.tile([128, 128], F32)
            make_identity(nc, ident[:])
            w1t = cpool.tile([D, D], F32)
            w2t = cpool.tile([D, D], F32)
            w3t = cpool.tile([D, 1], F32)
            gbt = cpool.tile([4, D], F32)
            b3t = cpool.tile([1, 1], F32)
            epst = cpool.tile([D, 1], F32)
            nc.sync.dma_start(w1t[:], w1[:])
            nc.sync.dma_start(w2t[:], w2[:])
            nc.sync.dma_start(w3t[:], w3[:])
            nc.sync.dma_start(gbt[:], gb[:])
            nc.sync.dma_start(b3t[:], b3[:])
            nc.vector.memset(epst[:], EPS)
            idxt = cpool.tile([128, sched.icols], I16)
            nc.sync.dma_start(idxt[:], idx[:])
            degs = cpool.tile([128, S], F32)
            nc.sync.dma_start(degs[:], degt[:])

            # gbT [64, 4] = (g1, bt1, g2, bt2) columns
            pgb = ppool1.tile([D, 4], F32, name="pgb")
            nc.tensor.transpose(pgb[:], gbt[:], ident[:4, :4])
            gbs = cpool.tile([D, 4], F32)
            nc.vector.tensor_copy(out=gbs[:], in_=pgb[:])

            dinv = cpool.tile([128, S], F32)
            nc.scalar.sqrt(dinv[:], degs[:])
            nc.vector.reciprocal(dinv[:], dinv[:])
            dinv_exp = cpool.tile([128, NPF], F32)
            for t in range(S):
                nc.vector.tensor_copy(
                    out=dinv_exp[:, t * 64:(t + 1) * 64],
                    in_=dinv[:, t:t + 1].to_broadcast([128, 64]))

            ident16 = cpool.tile([128, 128], F16)
            nc.vector.tensor_copy(out=ident16[:], in_=ident[:])
            hcast = cpool.tile([128, NPF], F16)
            acc = cpool.tile([128, NPF], F32)
            nc.sync.dma_start(acc[:], xl[:])        # borrow acc for x load
            nc.vector.tensor_mul(out=hcast[:], in0=acc[:], in1=dinv_exp[:])

            ag_in = dpool.tile([NP, D], F16, name="ag_in")
            # Shared (pair-HBM) collective output: NRT's fast path for
            # HBM-HBM AllGather. Safe across layers: an AllGather cannot
            # complete before every core has entered it, which is after the
            # sibling's previous-layer expand read of tabc finished.
            tabc = nc.dram_tensor("tabc", [NT, D], F16,
                                  addr_space="Shared").ap()
            tabs = [dpool.tile([NT, 2 * D], F16, name=f"tab{l}")
                    for l in range(3)]
            ar_in = dpool.tile([D, 2], F32, name="ar_in")
            ar_out = [dpool.tile([D, 2], F32, name=f"ar_out{l}")
                      for l in range(2)]

            yT = cpool.tile([D, NP], F32, name="yT")
            zT = cpool.tile([D, NP], F32, name="zT")
            st = cpool.tile([D, 2], F32, name="st")
            stg = cpool.tile([D, 2], F32, name="stg")
            scb = cpool.tile([D, 4], F32, name="scb")
            msq = cpool.tile([D, 1], F32, name="msq")
            rstd = cpool.tile([D, 1], F32, name="rstd")

            qi = 0
            for layer in range(3):
                # publish local slice; AllGather the compact fp16 table,
                # then expand to 256B-strided rows (the gather ISA's stride
                # granularity; the pad half of each row is never read).
                nc.sync.dma_start(
                    ag_in.rearrange("(p t) f -> p (t f)", t=S), hcast[:])
                if cfg.nc > 1:
                    nc.gpsimd.collective_compute(
                        "AllGather", mybir.AluOpType.bypass,
                        replica_groups=[list(range(cfg.nc))],
                        ins=[ag_in.opt()], outs=[tabc.opt()],
                    )
                else:  # single-core timing/sim stub
                    nc.sync.dma_start(tabc[0:NP, :], ag_in[:])
                tab = tabs[layer]
                # split the expand so A-half gathers start before the B half
                # of the table has been restrided
                nc.sync.dma_start(tab[0:cfg.h0_rows, 0:D],
                                  tabc[0:cfg.h0_rows, :])
                nc.sync.dma_start(tab[cfg.h0_rows:NT, 0:D],
                                  tabc[cfg.h0_rows:NT, :])
                tA = tab[0:cfg.h0_rows, 0:D]
                tB = tab[cfg.h0_rows:NT, 0:D]

                # Aggregation. Each half's columns form a flat stream staged
                # through fixed window tiles (WA/WB columns); 8-column
                # dma_gathers fill windows, chunks never straddling a window.
                # Per slot, identity matmuls (incl. the hcast self-loop)
                # accumulate the slot's stream ranges in PSUM; ACT drains
                # with the dinv scale. Pool/DMA/PE/ACT pipeline via pools.
                tiles = ({}, {})             # half -> window idx -> tile
                cursors = [0, 0]             # issued stream cols per half
                chunk_i = [0, 0]
                halves = ((tA, spool_a, WA, "stA"), (tB, spool_b, WB, "stB"))
                sa = sb = 0

                def emit_transpose(u):
                    # acc slot u -> f-major yT columns (PE + DVE)
                    pt = ppool.tile([D, 128], F32, tag="tp", name="pt")
                    nc.tensor.transpose(pt[:], acc[:, u * 64:(u + 1) * 64],
                                        ident[:])
                    nc.vector.tensor_copy(out=yT[:, u * 128:(u + 1) * 128],
                                          in_=pt[:])

                def emit_zblock(b):
                    # weight matmul over yT block b (512 cols = 4 slots)
                    c0_, c1_ = b * 512, min(NP, b * 512 + 512)
                    if layer < 2:
                        pz = ppool.tile([D, 512], F32, tag="pz", name="pz")
                        nc.tensor.matmul(pz[:, :c1_ - c0_],
                                         (w1t, w2t)[layer][:],
                                         yT[:, c0_:c1_], start=True, stop=True)
                        nc.vector.tensor_copy(out=zT[:, c0_:c1_],
                                              in_=pz[:, :c1_ - c0_])
                    else:
                        po = ppool.tile([1, 512], F32, tag="pz", name="po")
                        nc.tensor.matmul(po[:, :c1_ - c0_], w3t[:],
                                         yT[:, c0_:c1_], start=True, stop=True)
                        o_s = opool.tile([1, 512], F32, name="o_s")
                        nc.scalar.add(o_s[:, :c1_ - c0_], po[:, :c1_ - c0_],
                                      add=b3t[:])
                        nc.sync.dma_start(out[:, c0_:c1_], o_s[:, :c1_ - c0_])

                for t in range(S):
                    for half, need in ((0, sa + ca[t]), (1, sb + cb[t])):
                        base, pool, W, nm = halves[half]
                        while cursors[half] < need:
                            _, icol0, m, col0 = chunks_ab[half][chunk_i[half]]
                            chunk_i[half] += 1
                            w = col0 // W
                            if w not in tiles[half]:
                                tiles[half][w] = pool.tile([128, W * D], F16,
                                                           name=nm)
                            off = col0 - w * W
                            nc.gpsimd.dma_gather(
                                tiles[half][w][:, off * D:(off + m) * D]
                                .rearrange("p (m x) -> p m x", x=D),
                                base, idxt[:, icol0:icol0 + 8 * m],
                                128 * m, 128 * m, D, elem_step=2 * D,
                                single_packet=False, queue_num=qi % 4)
                            qi += 1
                            cursors[half] += m
                    pacc = apool.tile([128, D], F32, name="pacc")
                    nmm = ca[t] + cb[t]
                    nc.tensor.matmul(pacc[:], ident16[:],
                                     hcast[:, t*64:(t+1)*64],
                                     start=True, stop=(nmm == 0))
                    k = 0
                    for half, s0, cnt in ((0, sa, ca[t]), (1, sb, cb[t])):
                        W = halves[half][2]
                        for r in range(cnt):
                            j = s0 + r
                            k += 1
                            nc.tensor.matmul(
                                pacc[:], ident16[:],
                                tiles[half][j // W][:, (j % W) * D:
                                                    (j % W + 1) * D],
                                start=False, stop=(k == nmm))
                    sa += ca[t]
                    sb += cb[t]
                    # acc[:, t] = dinv[:, t] * psum   (ACT drain)
                    nc.scalar.activation(acc[:, t*64:(t+1)*64], pacc[:],
                                         AF.Copy, scale=dinv[:, t:t+1])
                    # interleave the previous slot's transpose (and, every 4
                    # slots, the weight matmul over the finished yT block)
                    # under the gather/accumulate pipeline, one slot delayed
                    # so the PE never stalls on the ACT drain.
                    if t >= 1:
                        emit_transpose(t - 1)
                        if (t - 1) % 4 == 3:
                            emit_zblock((t - 1) // 4)

                emit_transpose(S - 1)
                for b in range((S - 2) // 4 + 1, (NP + 511) // 512):
                    emit_zblock(b)

                if layer < 2:
                    # local BN sums: sum(z) and sum(z^2)  (pads are zero)
                    nc.scalar.activation(yT[:], zT[:], AF.Copy,
                                         accum_out=st[:, 0:1])
                    nc.scalar.activation(yT[:], zT[:], AF.Square,
                                         accum_out=st[:, 1:2])
                    nc.sync.dma_start(ar_in[:], st[:])
                    if cfg.nc > 1:
                        nc.gpsimd.collective_compute(
                            "AllReduce", mybir.AluOpType.add,
                            replica_groups=[list(range(cfg.nc))],
                            ins=[ar_in.opt()], outs=[ar_out[layer].opt()],
                        )
                    else:
                        nc.sync.dma_start(ar_out[layer][:], ar_in[:])
                    nc.sync.dma_start(stg[:], ar_out[layer][:])
                    nc.scalar.mul(scb[:, 0:1], stg[:, 0:1], inv_n)
                    nc.scalar.mul(scb[:, 1:2], stg[:, 1:2], inv_n)
                    nc.vector.tensor_mul(out=msq[:], in0=scb[:, 0:1],
                                         in1=scb[:, 0:1])
                    nc.vector.tensor_sub(out=scb[:, 1:2], in0=scb[:, 1:2],
                                         in1=msq[:])
                    # rstd = 1/sqrt(var+eps)
                    nc.scalar.activation(rstd[:], scb[:, 1:2], AF.Sqrt,
                                         bias=epst[:, 0:1])
                    nc.vector.reciprocal(rstd[:], rstd[:])
                    nc.vector.tensor_mul(out=scb[:, 2:3],
                                         in0=gbs[:, 2 * layer:2 * layer + 1],
                                         in1=rstd[:])
                    nc.vector.tensor_mul(out=msq[:], in0=scb[:, 0:1],
                                         in1=scb[:, 2:3])
                    nc.vector.tensor_sub(out=scb[:, 3:4],
                                         in0=gbs[:, 2 * layer + 1:2 * layer + 2],
                                         in1=msq[:])
                    # h.T = Relu(scale*z + bias); hloc = dinv * h (node-major)
                    nc.scalar.activation(yT[:], zT[:], AF.Relu,
                                         bias=scb[:, 3:4], scale=scb[:, 2:3])
                    for t in range(S):
                        ph = ppool.tile([128, D], F32, tag="tp", name="ph")
                        nc.tensor.transpose(ph[:], yT[:, t * 128:(t + 1) * 128],
                                            ident[:64, :64])
                        nc.vector.tensor_mul(
                            out=hcast[:, t * 64:(t + 1) * 64], in0=ph[:],
                            in1=dinv_exp[:, t * 64:(t + 1) * 64])
                    # pad rows self-zero: their dinv is ~1e-19 (deg=1e38)

    nc.compile()
    return nc


def make_in_maps(cfg, inputs, deg, perms, idxs):
    x = np.asarray(inputs["x"], dtype=np.float32)
    in_maps = []
    for c in range(cfg.nc):
        xp = np.zeros((cfg.npad, D), np.float32)
        xp[:cfg.nloc] = x[perms[c]]
        xlc = xp.reshape(cfg.slots, 128, D).transpose(1, 0, 2).reshape(128, -1)
        dg = np.full((cfg.npad,), 1e30, np.float32)
        dg[:cfg.nloc] = deg[perms[c]].astype(np.float32)
        dgt = dg.reshape(cfg.slots, 128).T.copy()
        in_maps.append({
            "xl": np.ascontiguousarray(xlc),
            "degt": np.ascontiguousarray(dgt),
            "idx": np.ascontiguousarray(idxs[c]),
            "w1": np.asarray(inputs["W1"], np.float32),
            "w2": np.asarray(inputs["W2"], np.float32),
            "w3": np.asarray(inputs["W3"], np.float32).reshape(D, 1),
            "gb": np.stack([
                np.asarray(inputs["g1"], np.float32),
                np.asarray(inputs["bt1"], np.float32),
                np.asarray(inputs["g2"], np.float32),
                np.asarray(inputs["bt2"], np.float32)]),
            "b3": np.asarray(inputs["b3"], np.float32).reshape(1, 1),
        })
    return in_maps


_CACHE = {}


def kernel(**inputs):
    cfg = Cfg(n_nodes=int(np.asarray(inputs["x"]).shape[0]), n_cores=NC)
    deg, perms, sched, idxs = host_prep(
        cfg, np.asarray(inputs["edge_index"]))

    key = (cfg.n, sched.key)
    if key not in _CACHE:
        _CACHE[key] = build(cfg, sched)
    nc = _CACHE[key]
    in_maps = make_in_maps(cfg, inputs, deg, perms, idxs)

    import concourse.bass_utils as bass_utils
    res = None
    for attempt in range(3):
        try:
            res = bass_utils.run_bass_kernel_spmd(
                nc, in_maps, core_ids=list(range(cfg.nc)))
            break
        except Exception:
            if attempt == 2:
                raise
    out = np.zeros((cfg.n,), np.float32)
    for c in range(cfg.nc):
        oc = np.asarray(res.results[c]["out"]).reshape(cfg.npad)
        out[perms[c]] = oc[:cfg.nloc]
    return out


# revision 25
# speedup vs baseline: 1.0317x; 1.0317x over previous
"""GCN regressor (3x GCNConv + BatchNorm + ReLU) on 8 Trainium2 NeuronCores.

Sharding (graph/data parallel, per the hint):
  - Nodes are split into 8 contiguous blocks of 6250; a core owns the dsts in
    its block and all edges pointing at them. Within a core, dsts are laid out
    rank i -> (lane p=i%128, slot t=i//128) in a [128, 49*64] layout (padded
    to 6272 rows); the rank order is chosen on host to minimize gather padding
    (window-sort by per-half in-degree).
  - Per layer the dinv-prescaled feature table (all 50176 padded rows) is
    AllGathered into every core's DRAM (the "halo exchange").
  - The sparse aggregation uses the SWDGE dma_gather custom instruction
    (GPSIMD mlp library): each instruction gathers up to 64*128 rows by int16
    index, amortizing the ~1us per-instruction SWDGE overhead over thousands
    of descriptors (~30 instructions/layer instead of 1600). int16 indices cap
    the addressable table at 32768 rows, so the 50176-row table is split in
    two halves (cores 0-4 / 5-7) with per-half gather streams; each lane's
    edges are split by source half, exhausted lanes point at a zero row.
  - Gathered rows are reduced per destination slot by identity-matmul
    accumulation in PSUM on the otherwise-idle PE engine; the self-loop term
    (hloc) is folded in as one extra matmul; the ACT engine drains PSUM with
    the dinv output scale. Gathers/PE/ACT pipeline across slot groups.
  - Dense math (x@W, BN, ReLU) runs on PE/DVE/ACT; BN statistics are
    AllReduced (biased variance, as the reference).
  - b1/b2 are absorbed by BatchNorm (BN(z+b)=BN(z)) and unused.

kernel(**inputs) takes FULL inputs, returns the FULL [50000] output (f32).
"""

import sys

sys.path.insert(0, '/opt/trn_rl_repo')

import numpy as np

import concourse.bass as bass
import concourse.bacc as bacc
import concourse.tile as tile
import concourse.mybir as mybir
from concourse import library_config
from concourse.masks import make_identity


def _patch_dma_gather():
    """Allow 128-byte gather payloads (elem_step stays 256B-granular, which
    the InstDMAGatherAnt stride_bytes_256 ISA field requires)."""
    import inspect, textwrap
    src = textwrap.dedent(inspect.getsource(bass.BassGpSimd.dma_gather))
    src = src.replace("""    assert (
        elem_size_bytes > 0 and elem_size_bytes % 256 == 0
    )  # transpose restriction""", "    assert elem_size_bytes > 0")
    g = dict(bass.BassGpSimd.dma_gather.__globals__)
    exec(src, g)
    bass.BassGpSimd.dma_gather = g["dma_gather"]


_patch_dma_gather()

F32 = mybir.dt.float32
F16 = mybir.dt.float16
I16 = mybir.dt.int16
AF = mybir.ActivationFunctionType

D = 64
NC = 8
EPS = 1e-5
H0_CORES = 5        # table halves: cores 0..4 / 5..7 (int16 idx < 32768)
WSORT = 768         # window size for the lane-order heuristic
MAXC = 64           # gather columns per dma_gather (single_packet=False
                    # lifts the 1024-idx single-packet ucode cap)
WA = 128            # A-half stage window (columns, multiple of MAXC)
WB = 64             # B-half stage window (columns)


class Cfg:
    def __init__(self, n_nodes, n_cores=NC):
        self.n = n_nodes
        self.nc = n_cores
        self.nloc = n_nodes // n_cores
        assert self.nloc * n_cores == n_nodes
        self.slots = self.nloc // 128 + 1          # ensures pad ranks exist
        self.npad = self.slots * 128
        assert self.nloc < self.npad
        self.ntab = self.npad * n_cores
        self.h0_rows = H0_CORES * self.npad
        self.h1_rows = self.ntab - self.h0_rows
        assert self.h0_rows < 32768 and self.h1_rows < 32768


class Sched:
    """Gather schedule: per-slot column counts and packed idx layout."""
    def __init__(self, ca, cb, chunks, icols):
        self.ca = ca            # [S] A-columns (H0) per slot
        self.cb = cb            # [S] B-columns (H1) per slot
        self.chunks = chunks    # list of (half, icol0, ncols) in issue order
        self.icols = icols      # total idx-tile int16 columns
        self.key = (tuple(ca), tuple(cb), tuple(chunks))


def host_prep(cfg, edge_index):
    """Degrees, per-core lane ordering, per-half gather idx streams."""
    n, nc_, nloc, npad, S = cfg.n, cfg.nc, cfg.nloc, cfg.npad, cfg.slots
    src = np.asarray(edge_index[0], dtype=np.int64)
    dst = np.asarray(edge_index[1], dtype=np.int64)
    deg = np.bincount(dst, minlength=n).astype(np.int64) + 1   # + self loop

    is_h1 = (src // nloc) >= H0_CORES
    e1 = np.bincount(dst[is_h1], minlength=n)
    e0 = np.bincount(dst[~is_h1], minlength=n)

    # lane order: sort by e0 desc in windows, e1 desc within each window
    perms = []
    rank = np.zeros(n, dtype=np.int64)
    for c in range(nc_):
        nodes = np.arange(c * nloc, (c + 1) * nloc)
        o = np.argsort(-e0[nodes], kind="stable")
        parts = []
        for s in range(0, nloc, WSORT):
            blk = o[s:s + WSORT]
            parts.append(blk[np.argsort(-e1[nodes[blk]], kind="stable")])
        p = nodes[np.concatenate(parts)]
        perms.append(p)
        rank[p] = np.arange(nloc)

    # table row of node m: core*npad + (rank%128)*S + rank//128
    g_of = (np.int64(npad) * (np.arange(n) // nloc)
            + (rank % 128) * S + rank // 128)
    zrel = 127 * S + (S - 1)   # pad row (rank npad-1) of a block, block-rel

    # global column counts (shared across cores so one program fits all)
    ca = np.zeros(S, dtype=np.int64)
    cb = np.zeros(S, dtype=np.int64)
    lane_lists = []   # per core: dict (t, p) -> (listA, listB) of table rows
    for c in range(nc_):
        m = (dst // nloc) == c
        s_c, d_c = src[m], dst[m]
        r_c = rank[d_c]
        h1_c = is_h1[m]
        order = np.lexsort((g_of[s_c], h1_c, r_c))  # rank, half, src row
        s_c, r_c, h1_c = s_c[order], r_c[order], h1_c[order]
        rows = g_of[s_c]
        # counts per rank and half
        cnt0 = np.bincount(r_c[~h1_c], minlength=npad)
        cnt1 = np.bincount(r_c[h1_c], minlength=npad)
        for t in range(S):
            ca[t] = max(ca[t], cnt0[t * 128:(t + 1) * 128].max())
            cb[t] = max(cb[t], cnt1[t * 128:(t + 1) * 128].max())
        lane_lists.append((rows[~h1_c], r_c[~h1_c], cnt0,
                           rows[h1_c] - cfg.h0_rows, r_c[h1_c], cnt1))

    # Per-half flat column streams (A then B), in slot order. Chunks are
    # fixed 8-column (1024-idx) pieces of each stream; stage windows are
    # WA/WB columns, multiples of 8, so chunks never straddle windows.
    chunks = []            # (half, icol0, ncols, stream_col0)
    icol = 0
    for half, cc in ((0, ca), (1, cb)):
        ncols = int(cc.sum())
        done = 0
        while done < ncols:
            m = min(MAXC, ncols - done)
            chunks.append((half, icol, m, done))
            icol += m * 8          # 128 idx/col / 16 partitions
            done += m

    idxs = np.zeros((nc_, 128, max(icol, 1)), dtype=np.int16)
    for c in range(nc_):
        rowsA, rA, cnt0, rowsB, rB, cnt1 = lane_lists[c]
        startsA = np.concatenate([[0], np.cumsum(cnt0)])
        startsB = np.concatenate([[0], np.cumsum(cnt1)])
        flat_cols = []     # int16 [128] per column: A stream then B stream
        for cc, rows_, starts, cnt in ((ca, rowsA, startsA, cnt0),
                                       (cb, rowsB, startsB, cnt1)):
            for t in range(S):
                ranks = t * 128 + np.arange(128)
                for k in range(int(cc[t])):
                    col = np.full(128, zrel, dtype=np.int64)
                    have = cnt[ranks] > k
                    col[have] = rows_[starts[ranks[have]] + k]
                    flat_cols.append(col.astype(np.int16))
        if flat_cols:
            fc = np.stack(flat_cols)            # [ncols, 128]
            flat = fc.reshape(-1)               # position k = col*128 + p
            wrapped = np.zeros((16, icol), np.int16)
            kk = np.arange(len(flat))
            wrapped[kk % 16, kk // 16] = flat
            idxs[c] = np.tile(wrapped, (8, 1))
    sched = Sched([int(x) for x in ca], [int(x) for x in cb], chunks,
                  max(icol, 1))
    return deg, perms, sched, idxs


def build(cfg, sched):
    nc = bacc.Bacc("TRN2", target_bir_lowering=False, debug=False,
                   enable_asserts=False, num_devices=cfg.nc,
                   num_swdge_queues=4)
    S = cfg.slots
    NPF = S * 64
    NT = cfg.ntab
    NP = cfg.npad
    ca, cb = sched.ca, sched.cb

    xl = nc.dram_tensor("xl", [128, NPF], F32, kind="ExternalInput").ap()
    degt = nc.dram_tensor("degt", [128, S], F32, kind="ExternalInput").ap()
    idx = nc.dram_tensor("idx", [128, sched.icols], I16,
                         kind="ExternalInput").ap()
    w1 = nc.dram_tensor("w1", [D, D], F32, kind="ExternalInput").ap()
    w2 = nc.dram_tensor("w2", [D, D], F32, kind="ExternalInput").ap()
    w3 = nc.dram_tensor("w3", [D, 1], F32, kind="ExternalInput").ap()
    gb = nc.dram_tensor("gb", [4, D], F32, kind="ExternalInput").ap()
    b3 = nc.dram_tensor("b3", [1, 1], F32, kind="ExternalInput").ap()
    out = nc.dram_tensor("out", [1, NP], F32, kind="ExternalOutput").ap()

    inv_n = 1.0 / cfg.n
    chunks_ab = ([c for c in sched.chunks if c[0] == 0],
                 [c for c in sched.chunks if c[0] == 1])

    with tile.TileContext(nc) as tc:
        with tc.tile_pool(name="const", bufs=1) as cpool, \
             tc.tile_pool(name="stga", bufs=2) as spool_a, \
             tc.tile_pool(name="stgb", bufs=2) as spool_b, \
             tc.tile_pool(name="osml", bufs=2) as opool, \
             tc.tile_pool(name="ps", bufs=2, space="PSUM") as ppool, \
             tc.tile_pool(name="acm", bufs=2, space="PSUM") as apool, \
             tc.tile_pool(name="ps1", bufs=1, space="PSUM") as ppool1, \
             tc.tile_pool(name="dram", bufs=1, space="DRAM") as dpool:

            nc.gpsimd.load_library(library_config.mlp)

            ident = cpool.tile([128, 128], F32)
            make_identity(nc, ident[:])
            w1t = cpool.tile([D, D], F32)
            w2t = cpool.tile([D, D], F32)
            w3t = cpool.tile([D, 1], F32)
            gbt = cpool.tile([4, D], F32)
            b3t = cpool.tile([1, 1], F32)
            epst = cpool.tile([D, 1], F32)
            nc.sync.dma_start(w1t[:], w1[:])
            nc.sync.dma_start(w2t[:], w2[:])
            nc.sync.dma_start(w3t[:], w3[:])
            nc.sync.dma_start(gbt[:], gb[:])
            nc.sync.dma_start(b3t[:], b3[:])
            nc.vector.memset(epst[:], EPS)
            idxt = cpool.tile([128, sched.icols], I16)
            nc.sync.dma_start(idxt[:], idx[:])
            degs = cpool.tile([128, S], F32)
            nc.sync.dma_start(degs[:], degt[:])

            # gbT [64, 4] = (g1, bt1, g2, bt2) columns
            pgb = ppool1.tile([D, 4], F32, name="pgb")
            nc.tensor.transpose(pgb[:], gbt[:], ident[:4, :4])
            gbs = cpool.tile([D, 4], F32)
            nc.vector.tensor_copy(out=gbs[:], in_=pgb[:])

            dinv = cpool.tile([128, S], F32)
            nc.scalar.sqrt(dinv[:], degs[:])
            nc.vector.reciprocal(dinv[:], dinv[:])
            dinv_exp = cpool.tile([128, NPF], F32)
            for t in range(S):
                nc.vector.tensor_copy(
                    out=dinv_exp[:, t * 64:(t + 1) * 64],
                    in_=dinv[:, t:t + 1].to_broadcast([128, 64]))

            ident16 = cpool.tile([128, 128], F16)
            nc.vector.tensor_copy(out=ident16[:], in_=ident[:])
            hcast = cpool.tile([128, NPF], F16)
            acc = cpool.tile([128, NPF], F32)
            nc.sync.dma_start(acc[:], xl[:])        # borrow acc for x load
            nc.vector.tensor_mul(out=hcast[:], in0=acc[:], in1=dinv_exp[:])

            ag_in = dpool.tile([NP, D], F16, name="ag_in")
            # Shared (pair-HBM) collective output: NRT's fast path for
            # HBM-HBM AllGather. Safe across layers: an AllGather cannot
            # complete before every core has entered it, which is after the
            # sibling's previous-layer expand read of tabc finished.
            tabc = nc.dram_tensor("tabc", [NT, D], F16,
                                  addr_space="Shared").ap()
            tabs = [dpool.tile([NT, 2 * D], F16, name=f"tab{l}")
                    for l in range(3)]
            ar_in = dpool.tile([D, 2], F32, name="ar_in")
            ar_out = [dpool.tile([NC * D, 2], F32, name=f"ar_out{l}")
                      for l in range(2)]

            yT = cpool.tile([D, NP], F32, name="yT")
            zT = cpool.tile([D, NP], F32, name="zT")
            st = cpool.tile([D, 2], F32, name="st")
            sta = cpool.tile([D, 16], F32, name="sta")
            stb = cpool.tile([D, 16], F32, name="stb")
            stw = cpool.tile([D, 16], F32, name="stw")
            stg = cpool.tile([D, 2], F32, name="stg")
            scb = cpool.tile([D, 4], F32, name="scb")
            msq = cpool.tile([D, 1], F32, name="msq")
            rstd = cpool.tile([D, 1], F32, name="rstd")

            qi = 0
            for layer in range(3):
                # publish local slice; AllGather the compact fp16 table,
                # then expand to 256B-strided rows (the gather ISA's stride
                # granularity; the pad half of each row is never read).
                nc.sync.dma_start(
                    ag_in.rearrange("(p t) f -> p (t f)", t=S), hcast[:])
                if cfg.nc > 1:
                    nc.gpsimd.collective_compute(
                        "AllGather", mybir.AluOpType.bypass,
                        replica_groups=[list(range(cfg.nc))],
                        ins=[ag_in.opt()], outs=[tabc.opt()],
                    )
                else:  # single-core timing/sim stub
                    nc.sync.dma_start(tabc[0:NP, :], ag_in[:])
                tab = tabs[layer]
                # split the expand so A-half gathers start before the B half
                # of the table has been restrided
                nc.sync.dma_start(tab[0:cfg.h0_rows, 0:D],
                                  tabc[0:cfg.h0_rows, :])
                nc.sync.dma_start(tab[cfg.h0_rows:NT, 0:D],
                                  tabc[cfg.h0_rows:NT, :])
                tA = tab[0:cfg.h0_rows, 0:D]
                tB = tab[cfg.h0_rows:NT, 0:D]

                # Aggregation. Each half's columns form a flat stream staged
                # through fixed window tiles (WA/WB columns); 8-column
                # dma_gathers fill windows, chunks never straddling a window.
                # Per slot, identity matmuls (incl. the hcast self-loop)
                # accumulate the slot's stream ranges in PSUM; ACT drains
                # with the dinv scale. Pool/DMA/PE/ACT pipeline via pools.
                tiles = ({}, {})             # half -> window idx -> tile
                cursors = [0, 0]             # issued stream cols per half
                chunk_i = [0, 0]
                halves = ((tA, spool_a, WA, "stA"), (tB, spool_b, WB, "stB"))
                sa = sb = 0

                def emit_transpose(u):
                    # acc slot u -> f-major yT columns (PE + DVE)
                    pt = ppool.tile([D, 128], F32, tag="tp", name="pt")
                    nc.tensor.transpose(pt[:], acc[:, u * 64:(u + 1) * 64],
                                        ident[:])
                    nc.vector.tensor_copy(out=yT[:, u * 128:(u + 1) * 128],
                                          in_=pt[:])

                def emit_zblock(b):
                    # weight matmul over yT block b (512 cols = 4 slots)
                    c0_, c1_ = b * 512, min(NP, b * 512 + 512)
                    if layer < 2:
                        pz = ppool.tile([D, 512], F32, tag="pz", name="pz")
                        nc.tensor.matmul(pz[:, :c1_ - c0_],
                                         (w1t, w2t)[layer][:],
                                         yT[:, c0_:c1_], start=True, stop=True)
                        nc.vector.tensor_copy(out=zT[:, c0_:c1_],
                                              in_=pz[:, :c1_ - c0_])
                        # per-block BN partial sums (yT block is dead until
                        # the ReLU pass rewrites it; reused as dummy out)
                        nc.scalar.activation(yT[:, c0_:c1_], zT[:, c0_:c1_],
                                             AF.Copy,
                                             accum_out=sta[:, b:b + 1])
                        nc.scalar.activation(yT[:, c0_:c1_], zT[:, c0_:c1_],
                                             AF.Square,
                                             accum_out=stb[:, b:b + 1])
                    else:
                        po = ppool.tile([1, 512], F32, tag="pz", name="po")
                        nc.tensor.matmul(po[:, :c1_ - c0_], w3t[:],
                                         yT[:, c0_:c1_], start=True, stop=True)
                        o_s = opool.tile([1, 512], F32, name="o_s")
                        nc.scalar.add(o_s[:, :c1_ - c0_], po[:, :c1_ - c0_],
                                      add=b3t[:])
                        nc.sync.dma_start(out[:, c0_:c1_], o_s[:, :c1_ - c0_])

                for t in range(S):
                    for half, need in ((0, sa + ca[t]), (1, sb + cb[t])):
                        base, pool, W, nm = halves[half]
                        while cursors[half] < need:
                            _, icol0, m, col0 = chunks_ab[half][chunk_i[half]]
                            chunk_i[half] += 1
                            w = col0 // W
                            if w not in tiles[half]:
                                tiles[half][w] = pool.tile([128, W * D], F16,
                                                           name=nm)
                            off = col0 - w * W
                            nc.gpsimd.dma_gather(
                                tiles[half][w][:, off * D:(off + m) * D]
                                .rearrange("p (m x) -> p m x", x=D),
                                base, idxt[:, icol0:icol0 + 8 * m],
                                128 * m, 128 * m, D, elem_step=2 * D,
                                single_packet=False, queue_num=qi % 4)
                            qi += 1
                            cursors[half] += m
                    pacc = apool.tile([128, D], F32, name="pacc")
                    nmm = ca[t] + cb[t]
                    nc.tensor.matmul(pacc[:], ident16[:],
                                     hcast[:, t*64:(t+1)*64],
                                     start=True, stop=(nmm == 0))
                    k = 0
                    for half, s0, cnt in ((0, sa, ca[t]), (1, sb, cb[t])):
                        W = halves[half][2]
                        for r in range(cnt):
                            j = s0 + r
                            k += 1
                            nc.tensor.matmul(
                                pacc[:], ident16[:],
                                tiles[half][j // W][:, (j % W) * D:
                                                    (j % W + 1) * D],
                                start=False, stop=(k == nmm))
                    sa += ca[t]
                    sb += cb[t]
                    # acc[:, t] = dinv[:, t] * psum   (ACT drain)
                    nc.scalar.activation(acc[:, t*64:(t+1)*64], pacc[:],
                                         AF.Copy, scale=dinv[:, t:t+1])
                    # interleave the previous slot's transpose (and, every 4
                    # slots, the weight matmul over the finished yT block)
                    # under the gather/accumulate pipeline, one slot delayed
                    # so the PE never stalls on the ACT drain.
                    if t >= 1:
                        emit_transpose(t - 1)
                        if (t - 1) % 4 == 3:
                            emit_zblock((t - 1) // 4)

                emit_transpose(S - 1)
                for b in range((S - 2) // 4 + 1, (NP + 511) // 512):
                    emit_zblock(b)

                if layer < 2:
                    # reduce the 13 per-block partials (pads are zero)
                    nb_ = (NP + 511) // 512
                    nc.scalar.activation(stw[:, 0:nb_], sta[:, 0:nb_],
                                         AF.Copy, accum_out=st[:, 0:1])
                    nc.scalar.activation(stw[:, 0:nb_], stb[:, 0:nb_],
                                         AF.Copy, accum_out=st[:, 1:2])
                    nc.sync.dma_start(ar_in[:], st[:])
                    # stats exchange as AllGather + local sum (cheaper than
                    # AllReduce for a tiny payload)
                    if cfg.nc > 1:
                        nc.gpsimd.collective_compute(
                            "AllGather", mybir.AluOpType.bypass,
                            replica_groups=[list(range(cfg.nc))],
                            ins=[ar_in.opt()], outs=[ar_out[layer].opt()],
                        )
                        nc.sync.dma_start(
                            stw[:].rearrange("d (c s) -> d c s", s=2),
                            ar_out[layer].rearrange("(c d) s -> d c s", d=D))
                        nc.vector.tensor_add(out=stw[:, 0:8], in0=stw[:, 0:8],
                                             in1=stw[:, 8:16])
                        nc.vector.tensor_add(out=stw[:, 0:4], in0=stw[:, 0:4],
                                             in1=stw[:, 4:8])
                        nc.vector.tensor_add(out=stg[:], in0=stw[:, 0:2],
                                             in1=stw[:, 2:4])
                    else:
                        nc.sync.dma_start(ar_out[layer][0:D, :], ar_in[:])
                        nc.sync.dma_start(stg[:], ar_out[layer][0:D, :])
                    nc.scalar.mul(scb[:, 0:1], stg[:, 0:1], inv_n)
                    nc.scalar.mul(scb[:, 1:2], stg[:, 1:2], inv_n)
                    nc.vector.tensor_mul(out=msq[:], in0=scb[:, 0:1],
                                         in1=scb[:, 0:1])
                    nc.vector.tensor_sub(out=scb[:, 1:2], in0=scb[:, 1:2],
                                         in1=msq[:])
                    # rstd = 1/sqrt(var+eps)
                    nc.scalar.activation(rstd[:], scb[:, 1:2], AF.Sqrt,
                                         bias=epst[:, 0:1])
                    nc.vector.reciprocal(rstd[:], rstd[:])
                    nc.vector.tensor_mul(out=scb[:, 2:3],
                                         in0=gbs[:, 2 * layer:2 * layer + 1],
                                         in1=rstd[:])
                    nc.vector.tensor_mul(out=msq[:], in0=scb[:, 0:1],
                                         in1=scb[:, 2:3])
                    nc.vector.tensor_sub(out=scb[:, 3:4],
                                         in0=gbs[:, 2 * layer + 1:2 * layer + 2],
                                         in1=msq[:])
                    # h.T = Relu(scale*z + bias); hloc = dinv * h (node-major)
                    nc.scalar.activation(yT[:], zT[:], AF.Relu,
                                         bias=scb[:, 3:4], scale=scb[:, 2:3])
                    for t in range(S):
                        ph = ppool.tile([128, D], F32, tag="tp", name="ph")
                        nc.tensor.transpose(ph[:], yT[:, t * 128:(t + 1) * 128],
                                            ident[:64, :64])
                        nc.vector.tensor_mul(
                            out=hcast[:, t * 64:(t + 1) * 64], in0=ph[:],
                            in1=dinv_exp[:, t * 64:(t + 1) * 64])
                    # pad rows self-zero: their dinv is ~1e-19 (deg=1e38)

    nc.compile()
    return nc


def make_in_maps(cfg, inputs, deg, perms, idxs):
    x = np.asarray(inputs["x"], dtype=np.float32)
    in_maps = []
    for c in range(cfg.nc):
        xp = np.zeros((cfg.npad, D), np.float32)
        xp[:cfg.nloc] = x[perms[c]]
        xlc = xp.reshape(cfg.slots, 128, D).transpose(1, 0, 2).reshape(128, -1)
        dg = np.full((cfg.npad,), 1e30, np.float32)
        dg[:cfg.nloc] = deg[perms[c]].astype(np.float32)
        dgt = dg.reshape(cfg.slots, 128).T.copy()
        in_maps.append({
            "xl": np.ascontiguousarray(xlc),
            "degt": np.ascontiguousarray(dgt),
            "idx": np.ascontiguousarray(idxs[c]),
            "w1": np.asarray(inputs["W1"], np.float32),
            "w2": np.asarray(inputs["W2"], np.float32),
            "w3": np.asarray(inputs["W3"], np.float32).reshape(D, 1),
            "gb": np.stack([
                np.asarray(inputs["g1"], np.float32),
                np.asarray(inputs["bt1"], np.float32),
                np.asarray(inputs["g2"], np.float32),
                np.asarray(inputs["bt2"], np.float32)]),
            "b3": np.asarray(inputs["b3"], np.float32).reshape(1, 1),
        })
    return in_maps


_CACHE = {}


def kernel(**inputs):
    cfg = Cfg(n_nodes=int(np.asarray(inputs["x"]).shape[0]), n_cores=NC)
    deg, perms, sched, idxs = host_prep(
        cfg, np.asarray(inputs["edge_index"]))

    key = (cfg.n, sched.key)
    if key not in _CACHE:
        _CACHE[key] = build(cfg, sched)
    nc = _CACHE[key]
    in_maps = make_in_maps(cfg, inputs, deg, perms, idxs)

    import concourse.bass_utils as bass_utils
    res = None
    for attempt in range(3):
        try:
            res = bass_utils.run_bass_kernel_spmd(
                nc, in_maps, core_ids=list(range(cfg.nc)))
            break
        except Exception:
            if attempt == 2:
                raise
    out = np.zeros((cfg.n,), np.float32)
    for c in range(cfg.nc):
        oc = np.asarray(res.results[c]["out"]).reshape(cfg.npad)
        out[perms[c]] = oc[:cfg.nloc]
    return out


# revision 26
# speedup vs baseline: 1.0393x; 1.0073x over previous
"""GCN regressor (3x GCNConv + BatchNorm + ReLU) on 8 Trainium2 NeuronCores.

Sharding (graph/data parallel, per the hint):
  - Nodes are split into 8 contiguous blocks of 6250; a core owns the dsts in
    its block and all edges pointing at them. Within a core, dsts are laid out
    rank i -> (lane p=i%128, slot t=i//128) in a [128, 49*64] layout (padded
    to 6272 rows); the rank order is chosen on host to minimize gather padding
    (window-sort by per-half in-degree).
  - Per layer the dinv-prescaled feature table (all 50176 padded rows) is
    AllGathered into every core's DRAM (the "halo exchange").
  - The sparse aggregation uses the SWDGE dma_gather custom instruction
    (GPSIMD mlp library): each instruction gathers up to 64*128 rows by int16
    index, amortizing the ~1us per-instruction SWDGE overhead over thousands
    of descriptors (~30 instructions/layer instead of 1600). int16 indices cap
    the addressable table at 32768 rows, so the 50176-row table is split in
    two halves (cores 0-4 / 5-7) with per-half gather streams; each lane's
    edges are split by source half, exhausted lanes point at a zero row.
  - Gathered rows are reduced per destination slot by identity-matmul
    accumulation in PSUM on the otherwise-idle PE engine; the self-loop term
    (hloc) is folded in as one extra matmul; the ACT engine drains PSUM with
    the dinv output scale. Gathers/PE/ACT pipeline across slot groups.
  - Dense math (x@W, BN, ReLU) runs on PE/DVE/ACT; BN statistics are
    AllReduced (biased variance, as the reference).
  - b1/b2 are absorbed by BatchNorm (BN(z+b)=BN(z)) and unused.

kernel(**inputs) takes FULL inputs, returns the FULL [50000] output (f32).
"""

import sys

sys.path.insert(0, '/opt/trn_rl_repo')

import numpy as np

import concourse.bass as bass
import concourse.bacc as bacc
import concourse.tile as tile
import concourse.mybir as mybir
from concourse import library_config
from concourse.masks import make_identity


def _patch_dma_gather():
    """Allow 128-byte gather payloads (elem_step stays 256B-granular, which
    the InstDMAGatherAnt stride_bytes_256 ISA field requires)."""
    import inspect, textwrap
    src = textwrap.dedent(inspect.getsource(bass.BassGpSimd.dma_gather))
    src = src.replace("""    assert (
        elem_size_bytes > 0 and elem_size_bytes % 256 == 0
    )  # transpose restriction""", "    assert elem_size_bytes > 0")
    g = dict(bass.BassGpSimd.dma_gather.__globals__)
    exec(src, g)
    bass.BassGpSimd.dma_gather = g["dma_gather"]


_patch_dma_gather()

F32 = mybir.dt.float32
F16 = mybir.dt.float16
I16 = mybir.dt.int16
AF = mybir.ActivationFunctionType

D = 64
NC = 8
EPS = 1e-5
H0_CORES = 5        # table halves: cores 0..4 / 5..7 (int16 idx < 32768)
WSORT = 768         # window size for the lane-order heuristic
MAXC = 64           # gather columns per dma_gather (single_packet=False
                    # lifts the 1024-idx single-packet ucode cap)
WA = 128            # A-half stage window (columns, multiple of MAXC)
WB = 64             # B-half stage window (columns)


class Cfg:
    def __init__(self, n_nodes, n_cores=NC):
        self.n = n_nodes
        self.nc = n_cores
        self.nloc = n_nodes // n_cores
        assert self.nloc * n_cores == n_nodes
        self.slots = self.nloc // 128 + 1          # ensures pad ranks exist
        self.npad = self.slots * 128
        assert self.nloc < self.npad
        self.ntab = self.npad * n_cores
        self.h0_rows = H0_CORES * self.npad
        self.h1_rows = self.ntab - self.h0_rows
        assert self.h0_rows < 32768 and self.h1_rows < 32768


class Sched:
    """Gather schedule: per-slot column counts and packed idx layout."""
    def __init__(self, ca, cb, chunks, icols):
        self.ca = ca            # [S] A-columns (H0) per slot
        self.cb = cb            # [S] B-columns (H1) per slot
        self.chunks = chunks    # list of (half, icol0, ncols) in issue order
        self.icols = icols      # total idx-tile int16 columns
        self.key = (tuple(ca), tuple(cb), tuple(chunks))


def host_prep(cfg, edge_index):
    """Degrees, per-core lane ordering, per-half gather idx streams."""
    n, nc_, nloc, npad, S = cfg.n, cfg.nc, cfg.nloc, cfg.npad, cfg.slots
    src = np.asarray(edge_index[0], dtype=np.int64)
    dst = np.asarray(edge_index[1], dtype=np.int64)
    deg = np.bincount(dst, minlength=n).astype(np.int64) + 1   # + self loop

    is_h1 = (src // nloc) >= H0_CORES
    e1 = np.bincount(dst[is_h1], minlength=n)
    e0 = np.bincount(dst[~is_h1], minlength=n)

    # lane order: sort by e0 desc in windows, e1 desc within each window
    perms = []
    rank = np.zeros(n, dtype=np.int64)
    for c in range(nc_):
        nodes = np.arange(c * nloc, (c + 1) * nloc)
        o = np.argsort(-e0[nodes], kind="stable")
        parts = []
        for s in range(0, nloc, WSORT):
            blk = o[s:s + WSORT]
            parts.append(blk[np.argsort(-e1[nodes[blk]], kind="stable")])
        p = nodes[np.concatenate(parts)]
        perms.append(p)
        rank[p] = np.arange(nloc)

    # table row of node m: core*npad + (rank%128)*S + rank//128
    g_of = (np.int64(npad) * (np.arange(n) // nloc)
            + (rank % 128) * S + rank // 128)
    zrel = 127 * S + (S - 1)   # pad row (rank npad-1) of a block, block-rel

    # global column counts (shared across cores so one program fits all)
    ca = np.zeros(S, dtype=np.int64)
    cb = np.zeros(S, dtype=np.int64)
    lane_lists = []   # per core: dict (t, p) -> (listA, listB) of table rows
    for c in range(nc_):
        m = (dst // nloc) == c
        s_c, d_c = src[m], dst[m]
        r_c = rank[d_c]
        h1_c = is_h1[m]
        order = np.lexsort((g_of[s_c], h1_c, r_c))  # rank, half, src row
        s_c, r_c, h1_c = s_c[order], r_c[order], h1_c[order]
        rows = g_of[s_c]
        # counts per rank and half
        cnt0 = np.bincount(r_c[~h1_c], minlength=npad)
        cnt1 = np.bincount(r_c[h1_c], minlength=npad)
        for t in range(S):
            ca[t] = max(ca[t], cnt0[t * 128:(t + 1) * 128].max())
            cb[t] = max(cb[t], cnt1[t * 128:(t + 1) * 128].max())
        lane_lists.append((rows[~h1_c], r_c[~h1_c], cnt0,
                           rows[h1_c] - cfg.h0_rows, r_c[h1_c], cnt1))

    # Per-half flat column streams (A then B), in slot order. Chunks are
    # fixed 8-column (1024-idx) pieces of each stream; stage windows are
    # WA/WB columns, multiples of 8, so chunks never straddle windows.
    chunks = []            # (half, icol0, ncols, stream_col0)
    icol = 0
    for half, cc in ((0, ca), (1, cb)):
        ncols = int(cc.sum())
        done = 0
        while done < ncols:
            m = min(MAXC, ncols - done)
            chunks.append((half, icol, m, done))
            icol += m * 8          # 128 idx/col / 16 partitions
            done += m

    idxs = np.zeros((nc_, 128, max(icol, 1)), dtype=np.int16)
    for c in range(nc_):
        rowsA, rA, cnt0, rowsB, rB, cnt1 = lane_lists[c]
        startsA = np.concatenate([[0], np.cumsum(cnt0)])
        startsB = np.concatenate([[0], np.cumsum(cnt1)])
        flat_cols = []     # int16 [128] per column: A stream then B stream
        for cc, rows_, starts, cnt in ((ca, rowsA, startsA, cnt0),
                                       (cb, rowsB, startsB, cnt1)):
            for t in range(S):
                ranks = t * 128 + np.arange(128)
                for k in range(int(cc[t])):
                    col = np.full(128, zrel, dtype=np.int64)
                    have = cnt[ranks] > k
                    col[have] = rows_[starts[ranks[have]] + k]
                    flat_cols.append(col.astype(np.int16))
        if flat_cols:
            fc = np.stack(flat_cols)            # [ncols, 128]
            flat = fc.reshape(-1)               # position k = col*128 + p
            wrapped = np.zeros((16, icol), np.int16)
            kk = np.arange(len(flat))
            wrapped[kk % 16, kk // 16] = flat
            idxs[c] = np.tile(wrapped, (8, 1))
    sched = Sched([int(x) for x in ca], [int(x) for x in cb], chunks,
                  max(icol, 1))
    return deg, perms, sched, idxs


def build(cfg, sched):
    nc = bacc.Bacc("TRN2", target_bir_lowering=False, debug=False,
                   enable_asserts=False, num_devices=cfg.nc,
                   num_swdge_queues=4)
    S = cfg.slots
    NPF = S * 64
    NT = cfg.ntab
    NP = cfg.npad
    ca, cb = sched.ca, sched.cb

    xl = nc.dram_tensor("xl", [128, NPF], F32, kind="ExternalInput").ap()
    degt = nc.dram_tensor("degt", [128, S], F32, kind="ExternalInput").ap()
    idx = nc.dram_tensor("idx", [128, sched.icols], I16,
                         kind="ExternalInput").ap()
    w1 = nc.dram_tensor("w1", [D, D], F32, kind="ExternalInput").ap()
    w2 = nc.dram_tensor("w2", [D, D], F32, kind="ExternalInput").ap()
    w3 = nc.dram_tensor("w3", [D, 1], F32, kind="ExternalInput").ap()
    gb = nc.dram_tensor("gb", [4, D], F32, kind="ExternalInput").ap()
    b3 = nc.dram_tensor("b3", [1, 1], F32, kind="ExternalInput").ap()
    out = nc.dram_tensor("out", [1, NP], F32, kind="ExternalOutput").ap()

    inv_n = 1.0 / cfg.n
    chunks_ab = ([c for c in sched.chunks if c[0] == 0],
                 [c for c in sched.chunks if c[0] == 1])

    with tile.TileContext(nc) as tc:
        with tc.tile_pool(name="const", bufs=1) as cpool, \
             tc.tile_pool(name="stga", bufs=2) as spool_a, \
             tc.tile_pool(name="stgb", bufs=2) as spool_b, \
             tc.tile_pool(name="osml", bufs=2) as opool, \
             tc.tile_pool(name="ps", bufs=2, space="PSUM") as ppool, \
             tc.tile_pool(name="acm", bufs=2, space="PSUM") as apool, \
             tc.tile_pool(name="ps1", bufs=1, space="PSUM") as ppool1, \
             tc.tile_pool(name="dram", bufs=1, space="DRAM") as dpool:

            nc.gpsimd.load_library(library_config.mlp)

            ident = cpool.tile([128, 128], F32)
            make_identity(nc, ident[:])
            w1t = cpool.tile([D, D], F32)
            w2t = cpool.tile([D, D], F32)
            w3t = cpool.tile([D, 1], F32)
            gbt = cpool.tile([4, D], F32)
            b3t = cpool.tile([1, 1], F32)
            epst = cpool.tile([D, 1], F32)
            nc.sync.dma_start(w1t[:], w1[:])
            nc.sync.dma_start(w2t[:], w2[:])
            nc.sync.dma_start(w3t[:], w3[:])
            nc.sync.dma_start(gbt[:], gb[:])
            nc.sync.dma_start(b3t[:], b3[:])
            nc.vector.memset(epst[:], EPS)
            idxt = cpool.tile([128, sched.icols], I16)
            nc.sync.dma_start(idxt[:], idx[:])
            degs = cpool.tile([128, S], F32)
            nc.sync.dma_start(degs[:], degt[:])

            # gbT [64, 4] = (g1, bt1, g2, bt2) columns
            pgb = ppool1.tile([D, 4], F32, name="pgb")
            nc.tensor.transpose(pgb[:], gbt[:], ident[:4, :4])
            gbs = cpool.tile([D, 4], F32)
            nc.vector.tensor_copy(out=gbs[:], in_=pgb[:])

            dinv = cpool.tile([128, S], F32)
            nc.scalar.sqrt(dinv[:], degs[:])
            nc.vector.reciprocal(dinv[:], dinv[:])
            dinv_exp = cpool.tile([128, NPF], F32)
            for t in range(S):
                nc.vector.tensor_copy(
                    out=dinv_exp[:, t * 64:(t + 1) * 64],
                    in_=dinv[:, t:t + 1].to_broadcast([128, 64]))

            ident16 = cpool.tile([128, 128], F16)
            nc.vector.tensor_copy(out=ident16[:], in_=ident[:])
            hcast = cpool.tile([128, NPF], F16)
            acc = cpool.tile([128, NPF], F32)
            nc.sync.dma_start(acc[:], xl[:])        # borrow acc for x load
            nc.vector.tensor_mul(out=hcast[:], in0=acc[:], in1=dinv_exp[:])

            ag_in = dpool.tile([NP, D], F16, name="ag_in")
            # Shared (pair-HBM) collective output: NRT's fast path for
            # HBM-HBM AllGather. Safe across layers: an AllGather cannot
            # complete before every core has entered it, which is after the
            # sibling's previous-layer expand read of tabc finished.
            tabc = nc.dram_tensor("tabc", [NT, D], F16,
                                  addr_space="Shared").ap()
            tabs = [dpool.tile([NT, 2 * D], F16, name=f"tab{l}")
                    for l in range(3)]
            ar_in = dpool.tile([D, 2], F32, name="ar_in")
            ar_out = [dpool.tile([NC * D, 2], F32, name=f"ar_out{l}")
                      for l in range(2)]

            yT = cpool.tile([D, NP], F32, name="yT")
            zT = cpool.tile([D, NP], F32, name="zT")
            st = cpool.tile([D, 2], F32, name="st")
            sta = cpool.tile([D, 16], F32, name="sta")
            stb = cpool.tile([D, 16], F32, name="stb")
            stw = cpool.tile([D, 16], F32, name="stw")
            stg = cpool.tile([D, 2], F32, name="stg")
            scb = cpool.tile([D, 4], F32, name="scb")
            msq = cpool.tile([D, 1], F32, name="msq")
            rstd = cpool.tile([D, 1], F32, name="rstd")

            qi = 0
            for layer in range(3):
                # publish local slice; AllGather the compact fp16 table,
                # then expand to 256B-strided rows (the gather ISA's stride
                # granularity; the pad half of each row is never read).
                nc.sync.dma_start(
                    ag_in.rearrange("(p t) f -> p (t f)", t=S), hcast[:])
                if cfg.nc > 1:
                    nc.gpsimd.collective_compute(
                        "AllGather", mybir.AluOpType.bypass,
                        replica_groups=[list(range(cfg.nc))],
                        ins=[ag_in.opt()], outs=[tabc.opt()],
                    )
                else:  # single-core timing/sim stub
                    nc.sync.dma_start(tabc[0:NP, :], ag_in[:])
                tab = tabs[layer]
                # split the expand so A-half gathers start before the B half
                # of the table has been restrided
                nc.sync.dma_start(tab[0:cfg.h0_rows, 0:D],
                                  tabc[0:cfg.h0_rows, :])
                nc.sync.dma_start(tab[cfg.h0_rows:NT, 0:D],
                                  tabc[cfg.h0_rows:NT, :])
                tA = tab[0:cfg.h0_rows, 0:D]
                tB = tab[cfg.h0_rows:NT, 0:D]

                # Aggregation. Each half's columns form a flat stream staged
                # through fixed window tiles (WA/WB columns); 8-column
                # dma_gathers fill windows, chunks never straddling a window.
                # Per slot, identity matmuls (incl. the hcast self-loop)
                # accumulate the slot's stream ranges in PSUM; ACT drains
                # with the dinv scale. Pool/DMA/PE/ACT pipeline via pools.
                tiles = ({}, {})             # half -> window idx -> tile
                cursors = [0, 0]             # issued stream cols per half
                chunk_i = [0, 0]
                halves = ((tA, spool_a, WA, "stA"), (tB, spool_b, WB, "stB"))
                sa = sb = 0

                def emit_transpose(u):
                    # acc slot u -> f-major yT columns (PE + DVE)
                    pt = ppool.tile([D, 128], F32, tag="tp", name="pt")
                    nc.tensor.transpose(pt[:], acc[:, u * 64:(u + 1) * 64],
                                        ident[:])
                    nc.vector.tensor_copy(out=yT[:, u * 128:(u + 1) * 128],
                                          in_=pt[:])

                def emit_zblock(b):
                    # weight matmul over yT block b (512 cols = 4 slots)
                    c0_, c1_ = b * 512, min(NP, b * 512 + 512)
                    if layer < 2:
                        pz = ppool.tile([D, 512], F32, tag="pz", name="pz")
                        nc.tensor.matmul(pz[:, :c1_ - c0_],
                                         (w1t, w2t)[layer][:],
                                         yT[:, c0_:c1_], start=True, stop=True)
                        nc.vector.tensor_copy(out=zT[:, c0_:c1_],
                                              in_=pz[:, :c1_ - c0_])
                        # per-block BN partial sums (yT block is dead until
                        # the ReLU pass rewrites it; reused as dummy out)
                        nc.scalar.activation(yT[:, c0_:c1_], zT[:, c0_:c1_],
                                             AF.Copy,
                                             accum_out=sta[:, b:b + 1])
                        nc.scalar.activation(yT[:, c0_:c1_], zT[:, c0_:c1_],
                                             AF.Square,
                                             accum_out=stb[:, b:b + 1])
                    else:
                        po = ppool.tile([1, 512], F32, tag="pz", name="po")
                        nc.tensor.matmul(po[:, :c1_ - c0_], w3t[:],
                                         yT[:, c0_:c1_], start=True, stop=True)
                        o_s = opool.tile([1, 512], F32, name="o_s")
                        nc.scalar.add(o_s[:, :c1_ - c0_], po[:, :c1_ - c0_],
                                      add=b3t[:])
                        nc.sync.dma_start(out[:, c0_:c1_], o_s[:, :c1_ - c0_])

                for t in range(S):
                    for half, need in ((0, sa + ca[t]), (1, sb + cb[t])):
                        base, pool, W, nm = halves[half]
                        while cursors[half] < need:
                            _, icol0, m, col0 = chunks_ab[half][chunk_i[half]]
                            chunk_i[half] += 1
                            w = col0 // W
                            if w not in tiles[half]:
                                tiles[half][w] = pool.tile([128, W * D], F16,
                                                           name=nm)
                            off = col0 - w * W
                            nc.gpsimd.dma_gather(
                                tiles[half][w][:, off * D:(off + m) * D]
                                .rearrange("p (m x) -> p m x", x=D),
                                base, idxt[:, icol0:icol0 + 8 * m],
                                128 * m, 128 * m, D, elem_step=2 * D,
                                single_packet=False, queue_num=qi % 4)
                            qi += 1
                            cursors[half] += m
                    pacc = apool.tile([128, D], F32, name="pacc")
                    nmm = ca[t] + cb[t]
                    nc.tensor.matmul(pacc[:], ident16[:],
                                     hcast[:, t*64:(t+1)*64],
                                     start=True, stop=(nmm == 0))
                    k = 0
                    for half, s0, cnt in ((0, sa, ca[t]), (1, sb, cb[t])):
                        W = halves[half][2]
                        for r in range(cnt):
                            j = s0 + r
                            k += 1
                            nc.tensor.matmul(
                                pacc[:], ident16[:],
                                tiles[half][j // W][:, (j % W) * D:
                                                    (j % W + 1) * D],
                                start=False, stop=(k == nmm))
                    sa += ca[t]
                    sb += cb[t]
                    # acc[:, t] = dinv[:, t] * psum   (ACT drain)
                    nc.scalar.activation(acc[:, t*64:(t+1)*64], pacc[:],
                                         AF.Copy, scale=dinv[:, t:t+1])
                    # interleave the previous slot's transpose (and, every 4
                    # slots, the weight matmul over the finished yT block)
                    # under the gather/accumulate pipeline, one slot delayed
                    # so the PE never stalls on the ACT drain.
                    if t >= 1:
                        emit_transpose(t - 1)
                        if (t - 1) % 4 == 3:
                            emit_zblock((t - 1) // 4)

                emit_transpose(S - 1)
                for b in range((S - 2) // 4 + 1, (NP + 511) // 512):
                    emit_zblock(b)

                if layer < 2:
                    # reduce the 13 per-block partials (pads are zero)
                    nb_ = (NP + 511) // 512
                    nc.scalar.activation(stw[:, 0:nb_], sta[:, 0:nb_],
                                         AF.Copy, accum_out=st[:, 0:1])
                    nc.scalar.activation(stw[:, 0:nb_], stb[:, 0:nb_],
                                         AF.Copy, accum_out=st[:, 1:2])
                    nc.sync.dma_start(ar_in[:], st[:])
                    # stats exchange as AllGather + local sum (cheaper than
                    # AllReduce for a tiny payload)
                    if cfg.nc > 1:
                        nc.gpsimd.collective_compute(
                            "AllGather", mybir.AluOpType.bypass,
                            replica_groups=[list(range(cfg.nc))],
                            ins=[ar_in.opt()], outs=[ar_out[layer].opt()],
                        )
                        nc.sync.dma_start(
                            stw[:].rearrange("d (c s) -> d c s", s=2),
                            ar_out[layer].rearrange("(c d) s -> d c s", d=D))
                        nc.vector.tensor_add(out=stw[:, 0:8], in0=stw[:, 0:8],
                                             in1=stw[:, 8:16])
                        nc.vector.tensor_add(out=stw[:, 0:4], in0=stw[:, 0:4],
                                             in1=stw[:, 4:8])
                        nc.vector.tensor_add(out=stg[:], in0=stw[:, 0:2],
                                             in1=stw[:, 2:4])
                    else:
                        nc.sync.dma_start(ar_out[layer][0:D, :], ar_in[:])
                        nc.sync.dma_start(stg[:], ar_out[layer][0:D, :])
                    nc.scalar.mul(scb[:, 0:1], stg[:, 0:1], inv_n)
                    nc.scalar.mul(scb[:, 1:2], stg[:, 1:2], inv_n)
                    nc.vector.tensor_mul(out=msq[:], in0=scb[:, 0:1],
                                         in1=scb[:, 0:1])
                    nc.vector.tensor_sub(out=scb[:, 1:2], in0=scb[:, 1:2],
                                         in1=msq[:])
                    # rstd = 1/sqrt(var+eps)
                    nc.scalar.activation(rstd[:], scb[:, 1:2], AF.Sqrt,
                                         bias=epst[:, 0:1])
                    nc.vector.reciprocal(rstd[:], rstd[:])
                    nc.vector.tensor_mul(out=scb[:, 2:3],
                                         in0=gbs[:, 2 * layer:2 * layer + 1],
                                         in1=rstd[:])
                    nc.vector.tensor_mul(out=msq[:], in0=scb[:, 0:1],
                                         in1=scb[:, 2:3])
                    nc.vector.tensor_sub(out=scb[:, 3:4],
                                         in0=gbs[:, 2 * layer + 1:2 * layer + 2],
                                         in1=msq[:])
                    # h.T = Relu(scale*z + bias); hcast = dinv * h, emitted
                    # per 512-col block so ACT (ReLU) pipelines with PE/DVE
                    # (back-transposes) instead of running serially.
                    for b_ in range((NP + 511) // 512):
                        c0_, c1_ = b_ * 512, min(NP, b_ * 512 + 512)
                        nc.scalar.activation(yT[:, c0_:c1_], zT[:, c0_:c1_],
                                             AF.Relu, bias=scb[:, 3:4],
                                             scale=scb[:, 2:3])
                        for t in range(c0_ // 128, c1_ // 128):
                            ph = ppool.tile([128, D], F32, tag="tp", name="ph")
                            nc.tensor.transpose(
                                ph[:], yT[:, t * 128:(t + 1) * 128],
                                ident[:64, :64])
                            nc.vector.tensor_mul(
                                out=hcast[:, t * 64:(t + 1) * 64], in0=ph[:],
                                in1=dinv_exp[:, t * 64:(t + 1) * 64])
                    # pad rows self-zero: their dinv is ~1e-19 (deg=1e38)

    nc.compile()
    return nc


def make_in_maps(cfg, inputs, deg, perms, idxs):
    x = np.asarray(inputs["x"], dtype=np.float32)
    in_maps = []
    for c in range(cfg.nc):
        xp = np.zeros((cfg.npad, D), np.float32)
        xp[:cfg.nloc] = x[perms[c]]
        xlc = xp.reshape(cfg.slots, 128, D).transpose(1, 0, 2).reshape(128, -1)
        dg = np.full((cfg.npad,), 1e30, np.float32)
        dg[:cfg.nloc] = deg[perms[c]].astype(np.float32)
        dgt = dg.reshape(cfg.slots, 128).T.copy()
        in_maps.append({
            "xl": np.ascontiguousarray(xlc),
            "degt": np.ascontiguousarray(dgt),
            "idx": np.ascontiguousarray(idxs[c]),
            "w1": np.asarray(inputs["W1"], np.float32),
            "w2": np.asarray(inputs["W2"], np.float32),
            "w3": np.asarray(inputs["W3"], np.float32).reshape(D, 1),
            "gb": np.stack([
                np.asarray(inputs["g1"], np.float32),
                np.asarray(inputs["bt1"], np.float32),
                np.asarray(inputs["g2"], np.float32),
                np.asarray(inputs["bt2"], np.float32)]),
            "b3": np.asarray(inputs["b3"], np.float32).reshape(1, 1),
        })
    return in_maps


_CACHE = {}


def kernel(**inputs):
    cfg = Cfg(n_nodes=int(np.asarray(inputs["x"]).shape[0]), n_cores=NC)
    deg, perms, sched, idxs = host_prep(
        cfg, np.asarray(inputs["edge_index"]))

    key = (cfg.n, sched.key)
    if key not in _CACHE:
        _CACHE[key] = build(cfg, sched)
    nc = _CACHE[key]
    in_maps = make_in_maps(cfg, inputs, deg, perms, idxs)

    import concourse.bass_utils as bass_utils
    res = None
    for attempt in range(3):
        try:
            res = bass_utils.run_bass_kernel_spmd(
                nc, in_maps, core_ids=list(range(cfg.nc)))
            break
        except Exception:
            if attempt == 2:
                raise
    out = np.zeros((cfg.n,), np.float32)
    for c in range(cfg.nc):
        oc = np.asarray(res.results[c]["out"]).reshape(cfg.npad)
        out[perms[c]] = oc[:cfg.nloc]
    return out


# revision 27
# speedup vs baseline: 1.0446x; 1.0051x over previous
"""GCN regressor (3x GCNConv + BatchNorm + ReLU) on 8 Trainium2 NeuronCores.

Sharding (graph/data parallel, per the hint):
  - Nodes are split into 8 contiguous blocks of 6250; a core owns the dsts in
    its block and all edges pointing at them. Within a core, dsts are laid out
    rank i -> (lane p=i%128, slot t=i//128) in a [128, 49*64] layout (padded
    to 6272 rows); the rank order is chosen on host to minimize gather padding
    (window-sort by per-half in-degree).
  - Per layer the dinv-prescaled feature table (all 50176 padded rows) is
    AllGathered into every core's DRAM (the "halo exchange").
  - The sparse aggregation uses the SWDGE dma_gather custom instruction
    (GPSIMD mlp library): each instruction gathers up to 64*128 rows by int16
    index, amortizing the ~1us per-instruction SWDGE overhead over thousands
    of descriptors (~30 instructions/layer instead of 1600). int16 indices cap
    the addressable table at 32768 rows, so the 50176-row table is split in
    two halves (cores 0-4 / 5-7) with per-half gather streams; each lane's
    edges are split by source half, exhausted lanes point at a zero row.
  - Gathered rows are reduced per destination slot by identity-matmul
    accumulation in PSUM on the otherwise-idle PE engine; the self-loop term
    (hloc) is folded in as one extra matmul; the ACT engine drains PSUM with
    the dinv output scale. Gathers/PE/ACT pipeline across slot groups.
  - Dense math (x@W, BN, ReLU) runs on PE/DVE/ACT; BN statistics are
    AllReduced (biased variance, as the reference).
  - b1/b2 are absorbed by BatchNorm (BN(z+b)=BN(z)) and unused.

kernel(**inputs) takes FULL inputs, returns the FULL [50000] output (f32).
"""

import sys

sys.path.insert(0, '/opt/trn_rl_repo')

import numpy as np

import concourse.bass as bass
import concourse.bacc as bacc
import concourse.tile as tile
import concourse.mybir as mybir
from concourse import library_config
from concourse.masks import make_identity


def _patch_dma_gather():
    """Allow 128-byte gather payloads (elem_step stays 256B-granular, which
    the InstDMAGatherAnt stride_bytes_256 ISA field requires)."""
    import inspect, textwrap
    src = textwrap.dedent(inspect.getsource(bass.BassGpSimd.dma_gather))
    src = src.replace("""    assert (
        elem_size_bytes > 0 and elem_size_bytes % 256 == 0
    )  # transpose restriction""", "    assert elem_size_bytes > 0")
    g = dict(bass.BassGpSimd.dma_gather.__globals__)
    exec(src, g)
    bass.BassGpSimd.dma_gather = g["dma_gather"]


_patch_dma_gather()

F32 = mybir.dt.float32
F16 = mybir.dt.float16
I16 = mybir.dt.int16
AF = mybir.ActivationFunctionType

D = 64
NC = 8
EPS = 1e-5
H0_CORES = 5        # table halves: cores 0..4 / 5..7 (int16 idx < 32768)
WSORT = 768         # window size for the lane-order heuristic
MAXC = 64           # gather columns per dma_gather (single_packet=False
                    # lifts the 1024-idx single-packet ucode cap)
WA = 128            # A-half stage window (columns, multiple of MAXC)
WB = 64             # B-half stage window (columns)


class Cfg:
    def __init__(self, n_nodes, n_cores=NC):
        self.n = n_nodes
        self.nc = n_cores
        self.nloc = n_nodes // n_cores
        assert self.nloc * n_cores == n_nodes
        self.slots = self.nloc // 128 + 1          # ensures pad ranks exist
        self.npad = self.slots * 128
        assert self.nloc < self.npad
        self.ntab = self.npad * n_cores
        self.h0_rows = H0_CORES * self.npad
        self.h1_rows = self.ntab - self.h0_rows
        assert self.h0_rows < 32768 and self.h1_rows < 32768


class Sched:
    """Gather schedule: per-slot column counts and packed idx layout."""
    def __init__(self, ca, cb, chunks, icols):
        self.ca = ca            # [S] A-columns (H0) per slot
        self.cb = cb            # [S] B-columns (H1) per slot
        self.chunks = chunks    # list of (half, icol0, ncols) in issue order
        self.icols = icols      # total idx-tile int16 columns
        self.key = (tuple(ca), tuple(cb), tuple(chunks))


def host_prep(cfg, edge_index):
    """Degrees, per-core lane ordering, per-half gather idx streams."""
    n, nc_, nloc, npad, S = cfg.n, cfg.nc, cfg.nloc, cfg.npad, cfg.slots
    src = np.asarray(edge_index[0], dtype=np.int64)
    dst = np.asarray(edge_index[1], dtype=np.int64)
    deg = np.bincount(dst, minlength=n).astype(np.int64) + 1   # + self loop

    is_h1 = (src // nloc) >= H0_CORES
    e1 = np.bincount(dst[is_h1], minlength=n)
    e0 = np.bincount(dst[~is_h1], minlength=n)

    # lane order: sort by e0 desc in windows, e1 desc within each window
    perms = []
    rank = np.zeros(n, dtype=np.int64)
    for c in range(nc_):
        nodes = np.arange(c * nloc, (c + 1) * nloc)
        o = np.argsort(-e0[nodes], kind="stable")
        parts = []
        for s in range(0, nloc, WSORT):
            blk = o[s:s + WSORT]
            parts.append(blk[np.argsort(-e1[nodes[blk]], kind="stable")])
        p = nodes[np.concatenate(parts)]
        perms.append(p)
        rank[p] = np.arange(nloc)

    # table row of node m: core*npad + (rank%128)*S + rank//128
    g_of = (np.int64(npad) * (np.arange(n) // nloc)
            + (rank % 128) * S + rank // 128)
    zrel = 127 * S + (S - 1)   # pad row (rank npad-1) of a block, block-rel

    # global column counts (shared across cores so one program fits all)
    ca = np.zeros(S, dtype=np.int64)
    cb = np.zeros(S, dtype=np.int64)
    lane_lists = []   # per core: dict (t, p) -> (listA, listB) of table rows
    for c in range(nc_):
        m = (dst // nloc) == c
        s_c, d_c = src[m], dst[m]
        r_c = rank[d_c]
        h1_c = is_h1[m]
        order = np.lexsort((g_of[s_c], h1_c, r_c))  # rank, half, src row
        s_c, r_c, h1_c = s_c[order], r_c[order], h1_c[order]
        rows = g_of[s_c]
        # counts per rank and half
        cnt0 = np.bincount(r_c[~h1_c], minlength=npad)
        cnt1 = np.bincount(r_c[h1_c], minlength=npad)
        for t in range(S):
            ca[t] = max(ca[t], cnt0[t * 128:(t + 1) * 128].max())
            cb[t] = max(cb[t], cnt1[t * 128:(t + 1) * 128].max())
        lane_lists.append((rows[~h1_c], r_c[~h1_c], cnt0,
                           rows[h1_c] - cfg.h0_rows, r_c[h1_c], cnt1))

    # Per-half flat column streams (A then B), in slot order. Chunks are
    # fixed 8-column (1024-idx) pieces of each stream; stage windows are
    # WA/WB columns, multiples of 8, so chunks never straddle windows.
    chunks = []            # (half, icol0, ncols, stream_col0)
    icol = 0
    for half, cc in ((0, ca), (1, cb)):
        ncols = int(cc.sum())
        done = 0
        while done < ncols:
            m = min(MAXC, ncols - done)
            chunks.append((half, icol, m, done))
            icol += m * 8          # 128 idx/col / 16 partitions
            done += m

    idxs = np.zeros((nc_, 128, max(icol, 1)), dtype=np.int16)
    for c in range(nc_):
        rowsA, rA, cnt0, rowsB, rB, cnt1 = lane_lists[c]
        startsA = np.concatenate([[0], np.cumsum(cnt0)])
        startsB = np.concatenate([[0], np.cumsum(cnt1)])
        flat_cols = []     # int16 [128] per column: A stream then B stream
        for cc, rows_, starts, cnt in ((ca, rowsA, startsA, cnt0),
                                       (cb, rowsB, startsB, cnt1)):
            for t in range(S):
                ranks = t * 128 + np.arange(128)
                for k in range(int(cc[t])):
                    col = np.full(128, zrel, dtype=np.int64)
                    have = cnt[ranks] > k
                    col[have] = rows_[starts[ranks[have]] + k]
                    flat_cols.append(col.astype(np.int16))
        if flat_cols:
            fc = np.stack(flat_cols)            # [ncols, 128]
            flat = fc.reshape(-1)               # position k = col*128 + p
            wrapped = np.zeros((16, icol), np.int16)
            kk = np.arange(len(flat))
            wrapped[kk % 16, kk // 16] = flat
            idxs[c] = np.tile(wrapped, (8, 1))
    sched = Sched([int(x) for x in ca], [int(x) for x in cb], chunks,
                  max(icol, 1))
    return deg, perms, sched, idxs


def build(cfg, sched):
    nc = bacc.Bacc("TRN2", target_bir_lowering=False, debug=False,
                   enable_asserts=False, num_devices=cfg.nc,
                   num_swdge_queues=4)
    S = cfg.slots
    NPF = S * 64
    NT = cfg.ntab
    NP = cfg.npad
    ca, cb = sched.ca, sched.cb

    xl = nc.dram_tensor("xl", [128, NPF], F16, kind="ExternalInput").ap()
    degt = nc.dram_tensor("degt", [128, S], F32, kind="ExternalInput").ap()
    idx = nc.dram_tensor("idx", [128, sched.icols], I16,
                         kind="ExternalInput").ap()
    w1 = nc.dram_tensor("w1", [D, D], F32, kind="ExternalInput").ap()
    w2 = nc.dram_tensor("w2", [D, D], F32, kind="ExternalInput").ap()
    w3 = nc.dram_tensor("w3", [D, 1], F32, kind="ExternalInput").ap()
    gb = nc.dram_tensor("gb", [4, D], F32, kind="ExternalInput").ap()
    b3 = nc.dram_tensor("b3", [1, 1], F32, kind="ExternalInput").ap()
    out = nc.dram_tensor("out", [1, NP], F32, kind="ExternalOutput").ap()

    inv_n = 1.0 / cfg.n
    chunks_ab = ([c for c in sched.chunks if c[0] == 0],
                 [c for c in sched.chunks if c[0] == 1])

    with tile.TileContext(nc) as tc:
        with tc.tile_pool(name="const", bufs=1) as cpool, \
             tc.tile_pool(name="stga", bufs=2) as spool_a, \
             tc.tile_pool(name="stgb", bufs=2) as spool_b, \
             tc.tile_pool(name="osml", bufs=2) as opool, \
             tc.tile_pool(name="ps", bufs=2, space="PSUM") as ppool, \
             tc.tile_pool(name="acm", bufs=2, space="PSUM") as apool, \
             tc.tile_pool(name="ps1", bufs=1, space="PSUM") as ppool1, \
             tc.tile_pool(name="dram", bufs=1, space="DRAM") as dpool:

            nc.gpsimd.load_library(library_config.mlp)

            ident = cpool.tile([128, 128], F32)
            make_identity(nc, ident[:])
            w1t = cpool.tile([D, D], F32)
            w2t = cpool.tile([D, D], F32)
            w3t = cpool.tile([D, 1], F32)
            gbt = cpool.tile([4, D], F32)
            b3t = cpool.tile([1, 1], F32)
            epst = cpool.tile([D, 1], F32)
            nc.sync.dma_start(w1t[:], w1[:])
            nc.sync.dma_start(w2t[:], w2[:])
            nc.sync.dma_start(w3t[:], w3[:])
            nc.sync.dma_start(gbt[:], gb[:])
            nc.sync.dma_start(b3t[:], b3[:])
            nc.vector.memset(epst[:], EPS)
            idxt = cpool.tile([128, sched.icols], I16)
            nc.sync.dma_start(idxt[:], idx[:])
            degs = cpool.tile([128, S], F32)
            nc.sync.dma_start(degs[:], degt[:])

            # gbT [64, 4] = (g1, bt1, g2, bt2) columns
            pgb = ppool1.tile([D, 4], F32, name="pgb")
            nc.tensor.transpose(pgb[:], gbt[:], ident[:4, :4])
            gbs = cpool.tile([D, 4], F32)
            nc.vector.tensor_copy(out=gbs[:], in_=pgb[:])

            dinv = cpool.tile([128, S], F32)
            nc.scalar.sqrt(dinv[:], degs[:])
            nc.vector.reciprocal(dinv[:], dinv[:])
            dinv_exp = cpool.tile([128, NPF], F32)
            for t in range(S):
                nc.vector.tensor_copy(
                    out=dinv_exp[:, t * 64:(t + 1) * 64],
                    in_=dinv[:, t:t + 1].to_broadcast([128, 64]))

            ident16 = cpool.tile([128, 128], F16)
            nc.vector.tensor_copy(out=ident16[:], in_=ident[:])
            hcast = cpool.tile([128, NPF], F16)
            acc = cpool.tile([128, NPF], F32)
            nc.sync.dma_start(hcast[:], xl[:])      # host pre-scaled x*dinv

            ag_in = dpool.tile([NP, D], F16, name="ag_in")
            # Shared (pair-HBM) collective output: NRT's fast path for
            # HBM-HBM AllGather. Safe across layers: an AllGather cannot
            # complete before every core has entered it, which is after the
            # sibling's previous-layer expand read of tabc finished.
            tabc = nc.dram_tensor("tabc", [NT, D], F16,
                                  addr_space="Shared").ap()
            tabs = [dpool.tile([NT, 2 * D], F16, name=f"tab{l}")
                    for l in range(3)]
            ar_in = dpool.tile([D, 2], F32, name="ar_in")
            ar_out = [dpool.tile([NC * D, 2], F32, name=f"ar_out{l}")
                      for l in range(2)]

            yT = cpool.tile([D, NP], F32, name="yT")
            zT = cpool.tile([D, NP], F32, name="zT")
            st = cpool.tile([D, 2], F32, name="st")
            sta = cpool.tile([D, 16], F32, name="sta")
            stb = cpool.tile([D, 16], F32, name="stb")
            stw = cpool.tile([D, 16], F32, name="stw")
            stg = cpool.tile([D, 2], F32, name="stg")
            scb = cpool.tile([D, 4], F32, name="scb")
            msq = cpool.tile([D, 1], F32, name="msq")
            rstd = cpool.tile([D, 1], F32, name="rstd")

            qi = 0
            for layer in range(3):
                # publish local slice; AllGather the compact fp16 table,
                # then expand to 256B-strided rows (the gather ISA's stride
                # granularity; the pad half of each row is never read).
                nc.sync.dma_start(
                    ag_in.rearrange("(p t) f -> p (t f)", t=S), hcast[:])
                if cfg.nc > 1:
                    nc.gpsimd.collective_compute(
                        "AllGather", mybir.AluOpType.bypass,
                        replica_groups=[list(range(cfg.nc))],
                        ins=[ag_in.opt()], outs=[tabc.opt()],
                    )
                else:  # single-core timing/sim stub
                    nc.sync.dma_start(tabc[0:NP, :], ag_in[:])
                tab = tabs[layer]
                # split the expand so A-half gathers start before the B half
                # of the table has been restrided
                nc.sync.dma_start(tab[0:cfg.h0_rows, 0:D],
                                  tabc[0:cfg.h0_rows, :])
                nc.sync.dma_start(tab[cfg.h0_rows:NT, 0:D],
                                  tabc[cfg.h0_rows:NT, :])
                tA = tab[0:cfg.h0_rows, 0:D]
                tB = tab[cfg.h0_rows:NT, 0:D]

                # Aggregation. Each half's columns form a flat stream staged
                # through fixed window tiles (WA/WB columns); 8-column
                # dma_gathers fill windows, chunks never straddling a window.
                # Per slot, identity matmuls (incl. the hcast self-loop)
                # accumulate the slot's stream ranges in PSUM; ACT drains
                # with the dinv scale. Pool/DMA/PE/ACT pipeline via pools.
                tiles = ({}, {})             # half -> window idx -> tile
                cursors = [0, 0]             # issued stream cols per half
                chunk_i = [0, 0]
                halves = ((tA, spool_a, WA, "stA"), (tB, spool_b, WB, "stB"))
                sa = sb = 0

                def emit_transpose(u):
                    # acc slot u -> f-major yT columns (PE + DVE)
                    pt = ppool.tile([D, 128], F32, tag="tp", name="pt")
                    nc.tensor.transpose(pt[:], acc[:, u * 64:(u + 1) * 64],
                                        ident[:])
                    nc.vector.tensor_copy(out=yT[:, u * 128:(u + 1) * 128],
                                          in_=pt[:])

                def emit_zblock(b):
                    # weight matmul over yT block b (512 cols = 4 slots)
                    c0_, c1_ = b * 512, min(NP, b * 512 + 512)
                    if layer < 2:
                        pz = ppool.tile([D, 512], F32, tag="pz", name="pz")
                        nc.tensor.matmul(pz[:, :c1_ - c0_],
                                         (w1t, w2t)[layer][:],
                                         yT[:, c0_:c1_], start=True, stop=True)
                        nc.vector.tensor_copy(out=zT[:, c0_:c1_],
                                              in_=pz[:, :c1_ - c0_])
                        # per-block BN partial sums (yT block is dead until
                        # the ReLU pass rewrites it; reused as dummy out)
                        nc.scalar.activation(yT[:, c0_:c1_], zT[:, c0_:c1_],
                                             AF.Copy,
                                             accum_out=sta[:, b:b + 1])
                        nc.scalar.activation(yT[:, c0_:c1_], zT[:, c0_:c1_],
                                             AF.Square,
                                             accum_out=stb[:, b:b + 1])
                    else:
                        po = ppool.tile([1, 512], F32, tag="pz", name="po")
                        nc.tensor.matmul(po[:, :c1_ - c0_], w3t[:],
                                         yT[:, c0_:c1_], start=True, stop=True)
                        o_s = opool.tile([1, 512], F32, name="o_s")
                        nc.scalar.add(o_s[:, :c1_ - c0_], po[:, :c1_ - c0_],
                                      add=b3t[:])
                        nc.sync.dma_start(out[:, c0_:c1_], o_s[:, :c1_ - c0_])

                for t in range(S):
                    for half, need in ((0, sa + ca[t]), (1, sb + cb[t])):
                        base, pool, W, nm = halves[half]
                        while cursors[half] < need:
                            _, icol0, m, col0 = chunks_ab[half][chunk_i[half]]
                            chunk_i[half] += 1
                            w = col0 // W
                            if w not in tiles[half]:
                                tiles[half][w] = pool.tile([128, W * D], F16,
                                                           name=nm)
                            off = col0 - w * W
                            nc.gpsimd.dma_gather(
                                tiles[half][w][:, off * D:(off + m) * D]
                                .rearrange("p (m x) -> p m x", x=D),
                                base, idxt[:, icol0:icol0 + 8 * m],
                                128 * m, 128 * m, D, elem_step=2 * D,
                                single_packet=False, queue_num=qi % 4)
                            qi += 1
                            cursors[half] += m
                    pacc = apool.tile([128, D], F32, name="pacc")
                    nmm = ca[t] + cb[t]
                    nc.tensor.matmul(pacc[:], ident16[:],
                                     hcast[:, t*64:(t+1)*64],
                                     start=True, stop=(nmm == 0))
                    k = 0
                    for half, s0, cnt in ((0, sa, ca[t]), (1, sb, cb[t])):
                        W = halves[half][2]
                        for r in range(cnt):
                            j = s0 + r
                            k += 1
                            nc.tensor.matmul(
                                pacc[:], ident16[:],
                                tiles[half][j // W][:, (j % W) * D:
                                                    (j % W + 1) * D],
                                start=False, stop=(k == nmm))
                    sa += ca[t]
                    sb += cb[t]
                    # acc[:, t] = dinv[:, t] * psum   (ACT drain)
                    nc.scalar.activation(acc[:, t*64:(t+1)*64], pacc[:],
                                         AF.Copy, scale=dinv[:, t:t+1])
                    # interleave the previous slot's transpose (and, every 4
                    # slots, the weight matmul over the finished yT block)
                    # under the gather/accumulate pipeline, one slot delayed
                    # so the PE never stalls on the ACT drain.
                    if t >= 1:
                        emit_transpose(t - 1)
                        if (t - 1) % 4 == 3:
                            emit_zblock((t - 1) // 4)

                emit_transpose(S - 1)
                for b in range((S - 2) // 4 + 1, (NP + 511) // 512):
                    emit_zblock(b)

                if layer < 2:
                    # reduce the 13 per-block partials (pads are zero)
                    nb_ = (NP + 511) // 512
                    nc.scalar.activation(stw[:, 0:nb_], sta[:, 0:nb_],
                                         AF.Copy, accum_out=st[:, 0:1])
                    nc.scalar.activation(stw[:, 0:nb_], stb[:, 0:nb_],
                                         AF.Copy, accum_out=st[:, 1:2])
                    nc.sync.dma_start(ar_in[:], st[:])
                    # stats exchange as AllGather + local sum (cheaper than
                    # AllReduce for a tiny payload)
                    if cfg.nc > 1:
                        nc.gpsimd.collective_compute(
                            "AllGather", mybir.AluOpType.bypass,
                            replica_groups=[list(range(cfg.nc))],
                            ins=[ar_in.opt()], outs=[ar_out[layer].opt()],
                        )
                        nc.sync.dma_start(
                            stw[:].rearrange("d (c s) -> d c s", s=2),
                            ar_out[layer].rearrange("(c d) s -> d c s", d=D))
                        nc.vector.tensor_add(out=stw[:, 0:8], in0=stw[:, 0:8],
                                             in1=stw[:, 8:16])
                        nc.vector.tensor_add(out=stw[:, 0:4], in0=stw[:, 0:4],
                                             in1=stw[:, 4:8])
                        nc.vector.tensor_add(out=stg[:], in0=stw[:, 0:2],
                                             in1=stw[:, 2:4])
                    else:
                        nc.sync.dma_start(ar_out[layer][0:D, :], ar_in[:])
                        nc.sync.dma_start(stg[:], ar_out[layer][0:D, :])
                    nc.scalar.mul(scb[:, 0:1], stg[:, 0:1], inv_n)
                    nc.scalar.mul(scb[:, 1:2], stg[:, 1:2], inv_n)
                    nc.vector.tensor_mul(out=msq[:], in0=scb[:, 0:1],
                                         in1=scb[:, 0:1])
                    nc.vector.tensor_sub(out=scb[:, 1:2], in0=scb[:, 1:2],
                                         in1=msq[:])
                    # rstd = 1/sqrt(var+eps)
                    nc.scalar.activation(rstd[:], scb[:, 1:2], AF.Sqrt,
                                         bias=epst[:, 0:1])
                    nc.vector.reciprocal(rstd[:], rstd[:])
                    nc.vector.tensor_mul(out=scb[:, 2:3],
                                         in0=gbs[:, 2 * layer:2 * layer + 1],
                                         in1=rstd[:])
                    nc.vector.tensor_mul(out=msq[:], in0=scb[:, 0:1],
                                         in1=scb[:, 2:3])
                    nc.vector.tensor_sub(out=scb[:, 3:4],
                                         in0=gbs[:, 2 * layer + 1:2 * layer + 2],
                                         in1=msq[:])
                    # h.T = Relu(scale*z + bias); hcast = dinv * h, emitted
                    # per 512-col block so ACT (ReLU) pipelines with PE/DVE
                    # (back-transposes) instead of running serially.
                    for b_ in range((NP + 511) // 512):
                        c0_, c1_ = b_ * 512, min(NP, b_ * 512 + 512)
                        nc.scalar.activation(yT[:, c0_:c1_], zT[:, c0_:c1_],
                                             AF.Relu, bias=scb[:, 3:4],
                                             scale=scb[:, 2:3])
                        for t in range(c0_ // 128, c1_ // 128):
                            ph = ppool.tile([128, D], F32, tag="tp", name="ph")
                            nc.tensor.transpose(
                                ph[:], yT[:, t * 128:(t + 1) * 128],
                                ident[:64, :64])
                            nc.vector.tensor_mul(
                                out=hcast[:, t * 64:(t + 1) * 64], in0=ph[:],
                                in1=dinv_exp[:, t * 64:(t + 1) * 64])
                    # pad rows self-zero: their dinv is ~1e-19 (deg=1e38)

    nc.compile()
    return nc


def make_in_maps(cfg, inputs, deg, perms, idxs):
    x = np.asarray(inputs["x"], dtype=np.float32)
    in_maps = []
    for c in range(cfg.nc):
        xp = np.zeros((cfg.npad, D), np.float32)
        xp[:cfg.nloc] = x[perms[c]] / np.sqrt(
            deg[perms[c]].astype(np.float32))[:, None]
        xlc = xp.reshape(cfg.slots, 128, D).transpose(1, 0, 2) \
                .reshape(128, -1).astype(np.float16)
        dg = np.full((cfg.npad,), 1e30, np.float32)
        dg[:cfg.nloc] = deg[perms[c]].astype(np.float32)
        dgt = dg.reshape(cfg.slots, 128).T.copy()
        in_maps.append({
            "xl": np.ascontiguousarray(xlc),
            "degt": np.ascontiguousarray(dgt),
            "idx": np.ascontiguousarray(idxs[c]),
            "w1": np.asarray(inputs["W1"], np.float32),
            "w2": np.asarray(inputs["W2"], np.float32),
            "w3": np.asarray(inputs["W3"], np.float32).reshape(D, 1),
            "gb": np.stack([
                np.asarray(inputs["g1"], np.float32),
                np.asarray(inputs["bt1"], np.float32),
                np.asarray(inputs["g2"], np.float32),
                np.asarray(inputs["bt2"], np.float32)]),
            "b3": np.asarray(inputs["b3"], np.float32).reshape(1, 1),
        })
    return in_maps


_CACHE = {}


def kernel(**inputs):
    cfg = Cfg(n_nodes=int(np.asarray(inputs["x"]).shape[0]), n_cores=NC)
    deg, perms, sched, idxs = host_prep(
        cfg, np.asarray(inputs["edge_index"]))

    key = (cfg.n, sched.key)
    if key not in _CACHE:
        _CACHE[key] = build(cfg, sched)
    nc = _CACHE[key]
    in_maps = make_in_maps(cfg, inputs, deg, perms, idxs)

    import concourse.bass_utils as bass_utils
    res = None
    for attempt in range(3):
        try:
            res = bass_utils.run_bass_kernel_spmd(
                nc, in_maps, core_ids=list(range(cfg.nc)))
            break
        except Exception:
            if attempt == 2:
                raise
    out = np.zeros((cfg.n,), np.float32)
    for c in range(cfg.nc):
        oc = np.asarray(res.results[c]["out"]).reshape(cfg.npad)
        out[perms[c]] = oc[:cfg.nloc]
    return out
